# revision 23
# baseline (speedup 1.0000x reference)
"""BiLSTM-CRF loss kernel for Trainium2 (8 NeuronCores, data-parallel over batch).

v3: merged-direction LSTM phase + separate pipelined CRF tail.
  - Both LSTM directions run interleaved in ONE loop: two independent
    dependency chains pipeline across PE/Act/DVE/Pool, hiding the per-step
    serial latency that bounded v2.
  - Hidden state written straight into per-direction h histories (bf16);
    emissions are assembled in batched 32-step blocks (4 matmuls + one
    scalar-engine bias pass per 128-col chunk) once both directions have
    covered the block - no per-step emission work at all.
  - Input projection per 4-step window into 1-bank PSUM slots (2 per
    direction, ping-pong); gate bias and the bwd -1e5 pad-kill folded in as
    rank-1 matmuls; recurrence matmuls accumulate in place (start=False).
  - Activations: one sigmoid over all 8 gate chunks per group (g-rows
    pre-doubled; tanh(x)=2*sigmoid(2x)-1 fixed in cell math) + tanh(c) per
    group; sigmoid+tanh share one act table -> no table loads in the loop.
  - Forward direction unmasked (pad suffix garbage is bounded and never
    read); backward masked via the -1e5 gate injection (h=c=0 exactly).
  - CRF beta recursion in exp space as a tail, 2 column-subgroups
    pipelined; exp(emit) batched per block (sigmoid never used in the tail
    -> one act-table load total); rescale every 8 steps via fp32 exponent
    bit extraction (no Ln / reciprocal).
"""

import numpy as np

PAD_IDX = 0
VOCAB, K, E, H = 30000, 20, 256, 256
B, T = 128, 512
NCORES = 8
BL = B // NCORES          # 16 sequences per core
NG = 2                    # batch groups per direction
GB = BL // NG
SLOTW = 8                 # slot capacity (steps); slot = 2 psum banks
BLK = 32                  # emit/exp/unary block size
NBLK = T // BLK
RESCALE = 8               # CRF rescale interval (beta steps, per subgroup)
NSUB = 4                  # CRF column subgroups
SB = BL // NSUB
NCH = 64                  # 128-token gather chunks

_cache = {}


def _build_program():
    from contextlib import ExitStack
    import concourse.bass as bass
    import concourse.bacc as bacc
    import concourse.tile as tile
    from concourse import mybir
    from concourse.masks import make_identity

    f32 = mybir.dt.float32
    i32 = mybir.dt.int32
    bf16 = mybir.dt.bfloat16
    u8 = mybir.dt.uint8
    AF = mybir.ActivationFunctionType
    OP = mybir.AluOpType

    nc = bacc.Bacc(None, target_bir_lowering=False, debug=False)
    names = {}

    with ExitStack() as ctx:
        tc = ctx.enter_context(tile.TileContext(nc))
        dram = ctx.enter_context(tc.tile_pool(name="dram", bufs=1, space="DRAM"))

        def din(key, shape, dt=f32):
            t = dram.tile(shape, dt, kind="ExternalInput", name=key)
            names[key] = t.tensor.name
            return t

        emb = din("emb", [VOCAB, E], bf16)
        toks = din("toks", [T * BL, 1], i32)
        masku = din("masku", [1, T * BL], u8)
        negm = din("negm", [1, T * BL], bf16)
        tags1f = din("tags1f", [K, T * BL], u8)
        tagsnx = din("tagsnx", [T * BL, K], u8)
        tagsfl = din("tagsfl", [T * BL, 1], i32)
        wih = {d: din(f"wih_{d}", [E, 4 * H], bf16) for d in "fb"}
        whh = {d: din(f"whh_{d}", [E, 4 * H], bf16) for d in "fb"}
        brow = {d: din(f"brow_{d}", [1, 4 * H], bf16) for d in "fb"}
        woutT = din("woutT", [4, 128, K], bf16)
        bout = din("bout", [K, 1])
        transT = din("transT", [K, K])
        trans = din("trans", [K, K])
        out_loss = dram.tile([1, BL], f32, kind="ExternalOutput")
        names["out"] = out_loss.tensor.name
        out_dbg = dram.tile([1, 4 * BL], f32, kind="ExternalOutput", name="out_dbg")
        names["dbg"] = out_dbg.tensor.name

        # PSUM: 4 window slots (1 bank each) + transpose + small tiles
        ps_slot = ctx.enter_context(tc.tile_pool(name="ps_slot", bufs=1, space="PSUM"))
        ps_t = ctx.enter_context(tc.tile_pool(name="ps_t", bufs=1, space="PSUM"))
        ps_s = ctx.enter_context(tc.tile_pool(name="ps_s", bufs=1, space="PSUM"))

        sg = ctx.enter_context(tc.tile_pool(name="sg", bufs=1))
        tmp = ctx.enter_context(tc.tile_pool(name="tmp", bufs=4))
        gat = ctx.enter_context(tc.tile_pool(name="gat", bufs=4))
        neg = ctx.enter_context(tc.tile_pool(name="neg", bufs=2))
        fin = ctx.enter_context(tc.tile_pool(name="fin", bufs=3))

        slots = [ps_slot.tile([128, 8, SLOTW, BL], f32, tag=f"slot{i}",
                              name=f"slot{i}") for i in range(3)]

        # window schedule: fwd = 64 x 8-step windows; bwd = 4-step head,
        # 63 x 8-step, 4-step tail -> boundaries stagger every 4 merged steps
        fwin = [(8 * i, 8) for i in range(64)]
        bwin = [(508, 4)] + [(500 - 8 * i, 8) for i in range(63)] + [(0, 4)]
        fslot = [0] * len(fwin)
        bslot = [0] * len(bwin)
        bslot[0], bslot[1] = 1, 2
        rot = [1, 0, 2]
        for k in range(126):
            sl = rot[k % 3]
            if k % 2 == 0:
                fslot[k // 2 + 1] = sl
            else:
                bslot[(k + 1) // 2 + 1] = sl
        # per-t lookup: (window index, toff)
        fmap = [None] * T
        for j, (t0, ln) in enumerate(fwin):
            for o in range(ln):
                fmap[t0 + o] = (j, o)
        bmap = [None] * T
        for j, (t0, ln) in enumerate(bwin):
            for o in range(ln):
                bmap[t0 + o] = (j, o)

        # ---- resident SBUF tensors ----
        s_wih = {d: sg.tile([128, 2, 4 * H], bf16, tag=f"wih{d}", name=f"wih{d}")
                 for d in "fb"}
        s_whh = {d: sg.tile([128, 2, 4 * H], bf16, tag=f"whh{d}", name=f"whh{d}")
                 for d in "fb"}
        s_brow = {d: sg.tile([1, 4 * H], bf16, tag=f"brow{d}", name=f"brow{d}")
                  for d in "fb"}
        for d in "fb":
            nc.sync.dma_start(out=s_wih[d][:], in_=wih[d][:].rearrange("(k p) m -> p k m", p=128))
            nc.sync.dma_start(out=s_whh[d][:], in_=whh[d][:].rearrange("(k p) m -> p k m", p=128))
            nc.sync.dma_start(out=s_brow[d][:], in_=brow[d][:])
        s_wout = sg.tile([128, 4, K], bf16, tag="wout")
        nc.sync.dma_start(out=s_wout[:], in_=woutT[:].rearrange("c p k -> p c k"))
        s_bout = sg.tile([K, 1], f32, tag="bout")
        nc.sync.dma_start(out=s_bout[:], in_=bout[:])
        s_transT = sg.tile([K, K], f32, tag="transT")
        nc.sync.dma_start(out=s_transT[:], in_=transT[:])
        s_expAT = sg.tile([K, K], f32, tag="expAT")
        nc.scalar.activation(s_expAT[:], s_transT[:], AF.Exp)

        onesb = sg.tile([1, 256], bf16, tag="onesb")
        nc.vector.memset(onesb[:], 1.0)
        onesrow = sg.tile([1, 128], bf16, tag="onesrow")
        nc.vector.memset(onesrow[:], 1.0)
        onesf = sg.tile([128, K], f32, tag="onesf")
        nc.vector.memset(onesf[:], 1.0)
        ident = sg.tile([128, 128], bf16, tag="ident")
        make_identity(nc, ident[:])

        c_mask = sg.tile([1, 1], i32, tag="c_mask")
        nc.vector.memset(c_mask[:], 0x7F800000)

        maskrep = sg.tile([128, T, BL], u8, tag="maskrep")
        nc.sync.dma_start(
            out=maskrep[:],
            in_=bass.AP(tensor=masku.tensor, offset=masku[:].offset,
                        ap=[[0, 128], [BL, T], [1, BL]]))

        idxall = sg.tile([128, NCH], i32, tag="idxall")
        nc.sync.dma_start(out=idxall[:],
                          in_=bass.AP(tensor=toks.tensor, offset=toks[:].offset,
                                      ap=[[1, 128], [128, NCH]]))
        idxtag = sg.tile([128, NCH], i32, tag="idxtag")
        nc.sync.dma_start(out=idxtag[:],
                          in_=bass.AP(tensor=tagsfl.tensor, offset=tagsfl[:].offset,
                                      ap=[[1, 128], [128, NCH]]))
        s_t1f = sg.tile([K, T * BL], u8, tag="s_t1f")
        nc.sync.dma_start(out=s_t1f[:], in_=tags1f[:])
        s_tnx = sg.tile([128, NCH, K], u8, tag="s_tnx")
        nc.sync.dma_start(out=s_tnx[:],
                          in_=tagsnx[:].rearrange("(n p) k -> p n k", p=128))

        xT = sg.tile([128, 2, T * BL], bf16, tag="xT")
        emit = sg.tile([K, T, BL], f32, tag="emit")
        expE = sg.tile([K, 2, BLK * BL], f32, tag="expE")
        hist = {d: sg.tile([128, 2, T, BL], bf16, tag=f"hist{d}", name=f"hist{d}")
                for d in "fb"}
        hzero = sg.tile([128, 2, BL], bf16, tag="hzero")
        nc.vector.memset(hzero[:], 0.0)

        st_c = {d: sg.tile([128, 2, BL], f32, tag=f"c{d}", name=f"c{d}") for d in "fb"}
        for d in "fb":
            nc.vector.memset(st_c[d][:], 0.0)

        Bv = sg.tile([K, BL], f32, tag="Bv")
        nc.vector.memset(Bv[:], 1.0)
        Eacc = sg.tile([1, BL], f32, tag="Eacc")
        nc.vector.memset(Eacc[:], 0.0)
        Uacc = sg.tile([K, BL], f32, tag="Uacc")
        nc.vector.memset(Uacc[:], 0.0)
        TRbuf = sg.tile([128, NCH], f32, tag="TRbuf")

        # ---- warm-up matmuls ----
        for wt in [s_wih["f"][:, 0, 0:1], s_wih["b"][:, 0, 0:1],
                   s_whh["f"][:, 0, 0:1], s_whh["b"][:, 0, 0:1],
                   s_wout[:, 0, 0:1], ident[:, 0:1]]:
            psd = ps_s.tile([1, 1], f32, tag="pssm", name="psd")
            nc.tensor.matmul(psd[:], lhsT=wt, rhs=wt, start=True, stop=True)
        psd = ps_s.tile([1, 1], f32, tag="pssm", name="psd")
        nc.tensor.matmul(psd[:], lhsT=s_expAT[0:K, 0:1], rhs=s_expAT[0:K, 0:1],
                         start=True, stop=True)

        # ---------------- background work queue ----------------
        bg_q = []

        def drain_bg(n):
            while n > 0 and bg_q:
                bg_q.pop(0)()
                n -= 1

        gathered = [0]

        def gather_chunk(c):
            def work():
                xg = gat.tile([128, E], bf16, tag="xg", name="xg")
                nc.gpsimd.indirect_dma_start(
                    out=xg[:], out_offset=None, in_=emb[:],
                    in_offset=bass.IndirectOffsetOnAxis(ap=idxall[:, c:c + 1], axis=0))
                for k in range(2):
                    pst = ps_t.tile([128, 128], bf16, tag="pst", name="pst")
                    nc.tensor.transpose(out=pst[:], in_=xg[:, k * 128:(k + 1) * 128],
                                        identity=ident[:])
                    nc.vector.tensor_copy(xT[:, k, c * 128:(c + 1) * 128], pst[:])
            return work

        def tag_chunk(i):
            def work():
                tr = gat.tile([128, K], f32, tag="tr", name="tr")
                nc.gpsimd.indirect_dma_start(
                    out=tr[:], out_offset=None, in_=trans[:],
                    in_offset=bass.IndirectOffsetOnAxis(ap=idxtag[:, i:i + 1], axis=0))
                sel = gat.tile([128, K], f32, tag="sel", name="sel")
                nc.vector.tensor_copy(sel[:], s_tnx[:, i, :])
                nc.vector.tensor_tensor(tr[:], tr[:], sel[:], op=OP.mult)
                nc.vector.tensor_reduce(TRbuf[:, i:i + 1], tr[:],
                                        axis=mybir.AxisListType.X, op=OP.add)
            return work

        # ---------------- window build pieces ----------------
        def negdma_piece(t0, ln, nm):
            c0, ncol = t0 * BL, ln * BL

            def work():
                nc.sync.dma_start(out=nm[:, 0:ncol], in_=negm[:, c0:c0 + ncol])
            return work

        def proj_piece(t0, ln, si, d, m, nm):
            c0, ncol = t0 * BL, ln * BL
            sl = slots[si]

            def work():
                o_m = bass.AP(tensor=sl.tensor,
                              offset=sl[:].offset + m * SLOTW * BL,
                              ap=[sl[:].ap[0], [1, ncol]])
                for k in range(2):
                    nc.tensor.matmul(o_m, lhsT=s_wih[d][:, k, m * 128:(m + 1) * 128],
                                     rhs=xT[:, k, c0:c0 + ncol],
                                     start=(m in (0, 4) and k == 0), stop=False,
                                     skip_group_check=True)
                nc.tensor.matmul(o_m, lhsT=s_brow[d][:, m * 128:(m + 1) * 128],
                                 rhs=onesb[:, 0:ncol], start=False, stop=False,
                                 skip_group_check=True)
                if d == "b" and m < 6:
                    nc.tensor.matmul(o_m, lhsT=onesrow[:, 0:128],
                                     rhs=nm[:, 0:ncol], start=False, stop=False,
                                     skip_group_check=True)
            return work

        def build_pieces(j, d):
            t0, ln = (fwin if d == "f" else bwin)[j]
            si = (fslot if d == "f" else bslot)[j]
            pieces = []
            nm = None
            if d == "b":
                nm = neg.tile([1, 256], bf16, tag="nm", name="nm")
                pieces.append(negdma_piece(t0, ln, nm))
            for m in range(8):
                pieces.append(proj_piece(t0, ln, si, d, m, nm))
            return pieces

        # ---------------- LSTM step ----------------
        def lstm_step(d, t):
            j, toff = (fmap if d == "f" else bmap)[t]
            sl = slots[(fslot if d == "f" else bslot)[j]]
            c = st_c[d]
            tprev = t - 1 if d == "f" else t + 1
            hin = hzero if (d == "f" and t == 0) or (d == "b" and t == T - 1) \
                else None
            for m in range(8):
                o_m = bass.AP(tensor=sl.tensor,
                              offset=sl[:].offset + (m * SLOTW + toff) * BL,
                              ap=[sl[:].ap[0], [1, BL]])
                for k in range(2):
                    rhs = hin[:, k, :] if hin is not None else hist[d][:, k, tprev, :]
                    nc.tensor.matmul(o_m, lhsT=s_whh[d][:, k, m * 128:(m + 1) * 128],
                                     rhs=rhs, start=False, stop=False,
                                     skip_group_check=True)
            for g in range(NG):
                cs = slice(g * GB, (g + 1) * GB)
                gin = bass.AP(tensor=sl.tensor,
                              offset=sl[:].offset + toff * BL + g * GB,
                              ap=[sl[:].ap[0], [SLOTW * BL, 8], [1, GB]])
                s = tmp.tile([128, 8, GB], f32, tag=f"s{d}{g}", name=f"s{d}{g}")
                nc.scalar.activation(s[:], gin, AF.Sigmoid)
                si, sf, so, sgt = s[:, 0:2], s[:, 2:4], s[:, 4:6], s[:, 6:8]
                ig = tmp.tile([128, 2, GB], f32, tag=f"ig{d}{g}", name=f"ig{d}{g}")
                nc.vector.scalar_tensor_tensor(ig[:], sgt, 0.5, si,
                                               op0=OP.subtract, op1=OP.mult)
                fc = tmp.tile([128, 2, GB], f32, tag=f"fc{d}{g}", name=f"fc{d}{g}")
                nc.gpsimd.tensor_tensor(fc[:], sf, c[:, :, cs], op=OP.mult)
                nc.vector.scalar_tensor_tensor(c[:, :, cs], ig[:], 2.0, fc[:],
                                               op0=OP.mult, op1=OP.add)
                th = tmp.tile([128, 2, GB], f32, tag=f"th{d}{g}", name=f"th{d}{g}")
                nc.scalar.activation(th[:], c[:, :, cs], AF.Tanh)
                nc.vector.tensor_tensor(hist[d][:, :, t, cs], so, th[:], op=OP.mult)

        # ---------------- emission block assembly ----------------
        def emit_chunk(blk, q):
            """emit[:, blk*32+q*8 : .. +8, :] = woutF@hf + woutB@hb + bias."""
            t0 = blk * BLK + q * 8
            n = 8 * BL

            def work():
                pe = ps_s.tile([K, n], f32, tag="pssm", name="pe")
                for ci, d in ((0, "f"), (2, "b")):
                    for k in range(2):
                        nc.tensor.matmul(
                            pe[:], lhsT=s_wout[:, ci + k, :],
                            rhs=hist[d][:, k, t0:t0 + 8, :].rearrange("p t b -> p (t b)"),
                            start=(ci == 0 and k == 0), stop=(ci == 2 and k == 1))
                nc.scalar.activation(
                    emit[:, t0:t0 + 8, :].rearrange("k t b -> k (t b)"),
                    pe[:], AF.Identity, bias=s_bout[:, 0:1])
            return work

        def unary_piece(blk):
            c0 = blk * BLK * BL
            n = BLK * BL

            def work():
                src = emit[:, blk * BLK:(blk + 1) * BLK, :].rearrange("k t b -> k (t b)")
                t1 = tmp.tile([K, n], f32, tag="t1", name="t1", bufs=1)
                nc.scalar.activation(t1[:], s_t1f[:, c0:c0 + n], AF.Identity)
                um = tmp.tile([K, n], f32, tag="um", name="um", bufs=1)
                nc.gpsimd.tensor_tensor(um[:], t1[:], src, op=OP.mult)
                ur = tmp.tile([K, BL], f32, tag="ur", name="ur")
                umr = bass.AP(tensor=um.tensor, offset=um[:].offset,
                              ap=[um[:].ap[0], [1, BL], [BL, BLK]])
                nc.vector.tensor_reduce(ur[:], umr, axis=mybir.AxisListType.X, op=OP.add)
                nc.vector.tensor_tensor(Uacc[:], Uacc[:], ur[:], op=OP.add)
            return work

        # ---------------- CRF beta tail ----------------
        rescale_count = [0]

        def exp_block(blk):
            src = emit[:, blk * BLK:(blk + 1) * BLK, :].rearrange("k t b -> k (t b)")
            nc.scalar.activation(expE[:, blk % 2, :], src, AF.Exp)

        def _slot_ps(si, col0, parts, ncols):
            sl = slots[si]
            return bass.AP(tensor=sl.tensor, offset=sl[:].offset + col0,
                           ap=[[sl[:].ap[0][0], parts], [1, ncols]])

        def beta_step(s, sub):
            cs = slice(sub * SB, (sub + 1) * SB)
            blk = (s + 1) // BLK
            col = ((s + 1) % BLK) * BL + sub * SB
            bp = tmp.tile([K, SB], f32, tag=f"bp{sub}", name=f"bp{sub}")
            nc.gpsimd.tensor_tensor(bp[:], Bv[:, cs], expE[:, blk % 2, col:col + SB],
                                    op=OP.mult)
            # per-subgroup PSUM bank (window slots are dead in the tail)
            psb = _slot_ps(sub // 2, (sub % 2) * 512, K, SB)
            nc.tensor.matmul(psb, lhsT=s_expAT[:], rhs=bp[:], start=True, stop=True,
                             skip_group_check=True)
            nc.vector.copy_predicated(Bv[:, cs], maskrep[0:K, s + 1, cs], psb)

        def beta_rescale(sub):
            cs = slice(sub * SB, (sub + 1) * SB)
            pss = _slot_ps(2, (sub % 2) * 512 + (sub // 2) * 64, 1, SB)
            nc.tensor.matmul(pss, lhsT=onesf[0:K, 0:1], rhs=Bv[:, cs],
                             start=True, stop=True, skip_group_check=True)
            em = tmp.tile([1, SB], i32, tag=f"em{sub}", name=f"em{sub}")
            nc.vector.tensor_scalar(em[:], pss.bitcast(i32), c_mask[:, 0:1],
                                    None, op0=OP.bitwise_and)
            ef = tmp.tile([1, SB], f32, tag=f"ef{sub}", name=f"ef{sub}")
            nc.vector.tensor_copy(ef[:], em[:])
            nc.vector.scalar_tensor_tensor(Eacc[:, cs], ef[:], 1.0 / (1 << 23),
                                           Eacc[:, cs], op0=OP.mult, op1=OP.add)
            scf = tmp.tile([1, SB], f32, tag=f"scf{sub}", name=f"scf{sub}")
            nc.vector.tensor_scalar(scf[:], ef[:], -1.0, float(0x7F000000),
                                    op0=OP.mult, op1=OP.add)
            sci = tmp.tile([1, SB], i32, tag=f"sci{sub}", name=f"sci{sub}")
            nc.vector.tensor_copy(sci[:], scf[:])
            psr = _slot_ps(2, (sub % 2) * 512 + (sub // 2) * 64 + 16, K, SB)
            nc.tensor.matmul(psr, lhsT=onesf[0:1, 0:K], rhs=sci[:].bitcast(f32),
                             start=True, stop=True, skip_group_check=True)
            nc.vector.tensor_tensor(Bv[:, cs], Bv[:, cs], psr, op=OP.mult)
            rescale_count[0] += 1

        # ================ merged LSTM phase ================
        misc_q = []

        def drain_misc(n):
            while n > 0 and misc_q:
                misc_q.pop(0)()
                n -= 1

        # token gather cursors: fwd consumes chunks ascending, bwd descending
        lo, hi = [0], [NCH - 1]

        def need_lo(upto_col):
            while lo[0] * 128 < upto_col and lo[0] <= hi[0]:
                bg_q.append(gather_chunk(lo[0]))
                lo[0] += 1

        def need_hi(from_col):
            while (hi[0] + 1) * 128 > from_col and hi[0] >= lo[0]:
                bg_q.append(gather_chunk(hi[0]))
                hi[0] -= 1

        # block assembly readiness: blk fully covered at merged step
        # s >= max(blk*BLK+BLK-1, T-1-blk*BLK)
        ready_at = {}
        for blk in range(NBLK):
            ready_at.setdefault(max(blk * BLK + BLK - 1, T - 1 - blk * BLK),
                                []).append(blk)

        # prime: tokens + first windows (fw0, bw0, bw1)
        need_lo(fwin[0][1] * BL)
        need_hi(bwin[1][0] * BL)
        for p in build_pieces(0, "f"):
            bg_q.append(p)
        for p in build_pieces(0, "b"):
            bg_q.append(p)
        for p in build_pieces(1, "b"):
            bg_q.append(p)
        drain_bg(len(bg_q))

        tag_i = [0]
        for s in range(T):
            tf, tb = s, T - 1 - s
            if s >= 4 and s % 4 == 0 and s <= 504:
                k = (s - 4) // 4
                if k % 2 == 0:
                    j = k // 2 + 1
                    need_lo((fwin[j][0] + fwin[j][1]) * BL)
                    for p in build_pieces(j, "f"):
                        bg_q.append(p)
                else:
                    j = (k + 1) // 2 + 1
                    need_hi(bwin[j][0] * BL)
                    for p in build_pieces(j, "b"):
                        bg_q.append(p)
            if s % 8 == 0 and tag_i[0] < NCH:
                misc_q.append(tag_chunk(tag_i[0]))
                tag_i[0] += 1
            lstm_step("f", tf)
            lstm_step("b", tb)
            for blk in ready_at.get(s, []):
                for q in range(4):
                    misc_q.append(emit_chunk(blk, q))
                misc_q.append(unary_piece(blk))
            drain_bg(3)
            drain_misc(1)
        drain_bg(len(bg_q))
        drain_misc(len(misc_q))

        # ================ CRF beta tail ================
        exp_block(NBLK - 1)
        for s in range(T - 2, -1, -1):
            if (s + 1) % BLK == BLK - 1:
                exp_block((s + 1) // BLK)
            for sub in range(NSUB):
                beta_step(s, sub)
            if s % RESCALE == 0 and s > 0:
                for sub in range(NSUB):
                    beta_rescale(sub)

        # ================ finalize ================
        zt = fin.tile([K, BL], f32, tag="zt")
        nc.vector.tensor_tensor(zt[:], Bv[:], expE[:, 0, 0:BL], op=OP.mult)
        psz = ps_s.tile([1, BL], f32, tag="pssm", name="psz")
        nc.tensor.matmul(psz[:], lhsT=onesf[0:K, 0:1], rhs=zt[:], start=True, stop=True)
        logZ = fin.tile([1, BL], f32, tag="logZ")
        nc.scalar.activation(logZ[:], psz[:], AF.Ln)
        nc.vector.scalar_tensor_tensor(logZ[:], Eacc[:], float(np.log(2.0)), logZ[:],
                                       op0=OP.mult, op1=OP.add)
        nc.vector.tensor_scalar(
            logZ[:], logZ[:],
            float(-127.0 * (rescale_count[0] // NSUB) * np.log(2.0)), None,
            op0=OP.add)

        psu = ps_s.tile([1, BL], f32, tag="pssm", name="psu")
        nc.tensor.matmul(psu[:], lhsT=onesf[0:K, 0:1], rhs=Uacc[:], start=True, stop=True)
        score = fin.tile([1, BL], f32, tag="score")
        nc.vector.tensor_copy(score[:], psu[:])

        QT = T // 128
        pstr = ps_s.tile([1, NCH], f32, tag="pssm", name="pstr")
        nc.tensor.matmul(pstr[:], lhsT=onesf[:, 0:1], rhs=TRbuf[:], start=True, stop=True)
        trv = fin.tile([1, BL], f32, tag="trv")
        ptr_ap = bass.AP(tensor=pstr.tensor, offset=pstr[:].offset,
                         ap=[pstr[:].ap[0], [QT, BL], [1, QT]])
        nc.vector.tensor_reduce(trv[:], ptr_ap, axis=mybir.AxisListType.X, op=OP.add)

        dbg = fin.tile([1, 4 * BL], f32, tag="dbg")
        nc.vector.tensor_copy(dbg[:, 0 * BL:1 * BL], score[:])
        nc.vector.tensor_copy(dbg[:, 1 * BL:2 * BL], trv[:])
        nc.vector.tensor_copy(dbg[:, 2 * BL:3 * BL], logZ[:])
        nc.vector.tensor_copy(dbg[:, 3 * BL:4 * BL], Eacc[:])
        nc.sync.dma_start(out=out_dbg[:], in_=dbg[:])

        nc.vector.tensor_tensor(score[:], score[:], trv[:], op=OP.add)
        res = fin.tile([1, BL], f32, tag="res")
        nc.vector.tensor_tensor(res[:], logZ[:], score[:], op=OP.subtract)
        nc.sync.dma_start(out=out_loss[:], in_=res[:])

    nc.compile()
    return nc, names


def _prep_core(inputs, core, perm):
    import ml_dtypes
    bf = ml_dtypes.bfloat16
    s = slice(core * BL, (core + 1) * BL)
    sent = np.asarray(inputs["sentences"][s])
    tags = np.asarray(inputs["tags"][s])
    mask = (sent != PAD_IDX)
    maskT = mask.T
    toks = np.ascontiguousarray(sent.T).reshape(T * BL, 1)
    oh = (tags[:, :, None] == np.arange(K)[None, None, :])
    tags1h = (oh & mask[:, :, None]).transpose(2, 1, 0).reshape(K, T * BL)
    tnx = np.zeros((BL, T, K), np.float32)
    tnx[:, :-1, :] = (oh[:, 1:, :] & mask[:, 1:, None]).astype(np.float32)

    def wprep(wname):
        wt = np.asarray(inputs[wname], np.float32)[perm].copy()
        wt[6 * 128:, :] *= 2.0
        return np.ascontiguousarray(wt.T).astype(bf)

    bvec = {}
    for d, key in (("f", "b_f"), ("b", "b_b")):
        bb = np.asarray(inputs[key], np.float32)[perm].copy()
        bb[6 * 128:] *= 2.0
        bvec[d] = bb.reshape(1, 4 * H).astype(bf)

    return {
        "toks": toks.astype(np.int32),
        "masku": maskT.astype(np.uint8).reshape(1, T * BL),
        "negm": ((~maskT).astype(np.float32) * -1e5).reshape(1, T * BL).astype(bf),
        "tags1f": tags1h.astype(np.uint8),
        "tagsnx": tnx.reshape(T * BL, K).astype(np.uint8),
        "tagsfl": tags.reshape(T * BL, 1).astype(np.int32),
        "emb": np.asarray(inputs["embedding"], np.float32).astype(bf),
        "wih_f": wprep("w_ih_f"), "wih_b": wprep("w_ih_b"),
        "whh_f": wprep("w_hh_f"), "whh_b": wprep("w_hh_b"),
        "brow_f": bvec["f"], "brow_b": bvec["b"],
        "woutT": np.ascontiguousarray(
            np.asarray(inputs["w_out"], np.float32).T.reshape(4, 128, K)).astype(bf),
        "bout": np.asarray(inputs["b_out"]).reshape(K, 1).astype(np.float32),
        "transT": np.ascontiguousarray(np.asarray(inputs["transition"]).T).astype(np.float32),
        "trans": np.asarray(inputs["transition"], np.float32),
    }


def kernel(**inputs):
    from concourse.bass_utils import run_bass_kernel_spmd

    if "prog" not in _cache:
        _cache["prog"] = _build_program()
    nc, names = _cache["prog"]

    blocks = np.arange(4 * H).reshape(4, H)
    perm = np.concatenate([blocks[0], blocks[1], blocks[3], blocks[2]])

    in_maps = []
    for core in range(NCORES):
        m = _prep_core(inputs, core, perm)
        in_maps.append({names[kk]: vv for kk, vv in m.items()})

    res = run_bass_kernel_spmd(nc, in_maps, core_ids=list(range(NCORES)),
                               **_cache.get("run_kwargs", {}))
    out = np.concatenate([r[names["out"]].reshape(BL) for r in res.results])
    _cache["last_results"] = res
    if "dbg" in names:
        _cache["dbg"] = np.concatenate(
            [r[names["dbg"]].reshape(4, BL) for r in res.results], axis=1)
    return out.astype(np.float32)


# revision 24
# speedup vs baseline: 1.0164x; 1.0164x over previous
"""BiLSTM-CRF loss kernel for Trainium2 (8 NeuronCores, data-parallel over batch).

v3: merged-direction LSTM phase + separate pipelined CRF tail.
  - Both LSTM directions run interleaved in ONE loop: two independent
    dependency chains pipeline across PE/Act/DVE/Pool, hiding the per-step
    serial latency that bounded v2.
  - Hidden state written straight into per-direction h histories (bf16);
    emissions are assembled in batched 32-step blocks (4 matmuls + one
    scalar-engine bias pass per 128-col chunk) once both directions have
    covered the block - no per-step emission work at all.
  - Input projection per 4-step window into 1-bank PSUM slots (2 per
    direction, ping-pong); gate bias and the bwd -1e5 pad-kill folded in as
    rank-1 matmuls; recurrence matmuls accumulate in place (start=False).
  - Activations: one sigmoid over all 8 gate chunks per group (g-rows
    pre-doubled; tanh(x)=2*sigmoid(2x)-1 fixed in cell math) + tanh(c) per
    group; sigmoid+tanh share one act table -> no table loads in the loop.
  - Forward direction unmasked (pad suffix garbage is bounded and never
    read); backward masked via the -1e5 gate injection (h=c=0 exactly).
  - CRF beta recursion in exp space as a tail, 2 column-subgroups
    pipelined; exp(emit) batched per block (sigmoid never used in the tail
    -> one act-table load total); rescale every 8 steps via fp32 exponent
    bit extraction (no Ln / reciprocal).
"""

import numpy as np

PAD_IDX = 0
VOCAB, K, E, H = 30000, 20, 256, 256
B, T = 128, 512
NCORES = 8
BL = B // NCORES          # 16 sequences per core
NG = 2                    # batch groups per direction
GB = BL // NG
SLOTW = 8                 # slot capacity (steps); slot = 2 psum banks
BLK = 32                  # emit/exp/unary block size
NBLK = T // BLK
RESCALE = 8               # CRF rescale interval (beta steps, per subgroup)
NSUB = 2                  # CRF column subgroups
SB = BL // NSUB
NCH = 64                  # 128-token gather chunks

_cache = {}


def _build_program():
    from contextlib import ExitStack
    import concourse.bass as bass
    import concourse.bacc as bacc
    import concourse.tile as tile
    from concourse import mybir
    from concourse.masks import make_identity

    f32 = mybir.dt.float32
    i32 = mybir.dt.int32
    bf16 = mybir.dt.bfloat16
    u8 = mybir.dt.uint8
    AF = mybir.ActivationFunctionType
    OP = mybir.AluOpType

    nc = bacc.Bacc(None, target_bir_lowering=False, debug=False)
    names = {}

    with ExitStack() as ctx:
        tc = ctx.enter_context(tile.TileContext(nc))
        dram = ctx.enter_context(tc.tile_pool(name="dram", bufs=1, space="DRAM"))

        def din(key, shape, dt=f32):
            t = dram.tile(shape, dt, kind="ExternalInput", name=key)
            names[key] = t.tensor.name
            return t

        emb = din("emb", [VOCAB, E], bf16)
        toks = din("toks", [T * BL, 1], i32)
        masku = din("masku", [1, T * BL], u8)
        negm = din("negm", [1, T * BL], bf16)
        tags1f = din("tags1f", [K, T * BL], u8)
        tagsnx = din("tagsnx", [T * BL, K], u8)
        tagsfl = din("tagsfl", [T * BL, 1], i32)
        wih = {d: din(f"wih_{d}", [E, 4 * H], bf16) for d in "fb"}
        whh = {d: din(f"whh_{d}", [E, 4 * H], bf16) for d in "fb"}
        brow = {d: din(f"brow_{d}", [1, 4 * H], bf16) for d in "fb"}
        woutT = din("woutT", [4, 128, K], bf16)
        bout = din("bout", [K, 1])
        transT = din("transT", [K, K])
        trans = din("trans", [K, K])
        out_loss = dram.tile([1, BL], f32, kind="ExternalOutput")
        names["out"] = out_loss.tensor.name
        out_dbg = dram.tile([1, 4 * BL], f32, kind="ExternalOutput", name="out_dbg")
        names["dbg"] = out_dbg.tensor.name

        # PSUM: 4 window slots (1 bank each) + transpose + small tiles
        ps_slot = ctx.enter_context(tc.tile_pool(name="ps_slot", bufs=1, space="PSUM"))
        ps_t = ctx.enter_context(tc.tile_pool(name="ps_t", bufs=1, space="PSUM"))
        ps_s = ctx.enter_context(tc.tile_pool(name="ps_s", bufs=1, space="PSUM"))

        sg = ctx.enter_context(tc.tile_pool(name="sg", bufs=1))
        tmp = ctx.enter_context(tc.tile_pool(name="tmp", bufs=4))
        gat = ctx.enter_context(tc.tile_pool(name="gat", bufs=4))
        neg = ctx.enter_context(tc.tile_pool(name="neg", bufs=2))
        fin = ctx.enter_context(tc.tile_pool(name="fin", bufs=3))

        slots = [ps_slot.tile([128, 8, SLOTW, BL], f32, tag=f"slot{i}",
                              name=f"slot{i}") for i in range(3)]

        # window schedule: fwd = 64 x 8-step windows; bwd = 4-step head,
        # 63 x 8-step, 4-step tail -> boundaries stagger every 4 merged steps
        fwin = [(8 * i, 8) for i in range(64)]
        bwin = [(508, 4)] + [(500 - 8 * i, 8) for i in range(63)] + [(0, 4)]
        fslot = [0] * len(fwin)
        bslot = [0] * len(bwin)
        bslot[0], bslot[1] = 1, 2
        rot = [1, 0, 2]
        for k in range(126):
            sl = rot[k % 3]
            if k % 2 == 0:
                fslot[k // 2 + 1] = sl
            else:
                bslot[(k + 1) // 2 + 1] = sl
        # per-t lookup: (window index, toff)
        fmap = [None] * T
        for j, (t0, ln) in enumerate(fwin):
            for o in range(ln):
                fmap[t0 + o] = (j, o)
        bmap = [None] * T
        for j, (t0, ln) in enumerate(bwin):
            for o in range(ln):
                bmap[t0 + o] = (j, o)

        # ---- resident SBUF tensors ----
        s_wih = {d: sg.tile([128, 2, 4 * H], bf16, tag=f"wih{d}", name=f"wih{d}")
                 for d in "fb"}
        s_whh = {d: sg.tile([128, 2, 4 * H], bf16, tag=f"whh{d}", name=f"whh{d}")
                 for d in "fb"}
        s_brow = {d: sg.tile([1, 4 * H], bf16, tag=f"brow{d}", name=f"brow{d}")
                  for d in "fb"}
        for d in "fb":
            nc.sync.dma_start(out=s_wih[d][:], in_=wih[d][:].rearrange("(k p) m -> p k m", p=128))
            nc.sync.dma_start(out=s_whh[d][:], in_=whh[d][:].rearrange("(k p) m -> p k m", p=128))
            nc.sync.dma_start(out=s_brow[d][:], in_=brow[d][:])
        s_wout = sg.tile([128, 4, K], bf16, tag="wout")
        nc.sync.dma_start(out=s_wout[:], in_=woutT[:].rearrange("c p k -> p c k"))
        s_bout = sg.tile([K, 1], f32, tag="bout")
        nc.sync.dma_start(out=s_bout[:], in_=bout[:])
        s_transT = sg.tile([K, K], f32, tag="transT")
        nc.sync.dma_start(out=s_transT[:], in_=transT[:])
        s_expAT = sg.tile([K, K], f32, tag="expAT")
        nc.scalar.activation(s_expAT[:], s_transT[:], AF.Exp)

        onesb = sg.tile([1, 256], bf16, tag="onesb")
        nc.vector.memset(onesb[:], 1.0)
        onesrow = sg.tile([1, 128], bf16, tag="onesrow")
        nc.vector.memset(onesrow[:], 1.0)
        onesf = sg.tile([128, K], f32, tag="onesf")
        nc.vector.memset(onesf[:], 1.0)
        ident = sg.tile([128, 128], bf16, tag="ident")
        make_identity(nc, ident[:])

        c_mask = sg.tile([1, 1], i32, tag="c_mask")
        nc.vector.memset(c_mask[:], 0x7F800000)

        maskrep = sg.tile([128, T, BL], u8, tag="maskrep")
        nc.sync.dma_start(
            out=maskrep[:],
            in_=bass.AP(tensor=masku.tensor, offset=masku[:].offset,
                        ap=[[0, 128], [BL, T], [1, BL]]))

        idxall = sg.tile([128, NCH], i32, tag="idxall")
        nc.sync.dma_start(out=idxall[:],
                          in_=bass.AP(tensor=toks.tensor, offset=toks[:].offset,
                                      ap=[[1, 128], [128, NCH]]))
        idxtag = sg.tile([128, NCH], i32, tag="idxtag")
        nc.sync.dma_start(out=idxtag[:],
                          in_=bass.AP(tensor=tagsfl.tensor, offset=tagsfl[:].offset,
                                      ap=[[1, 128], [128, NCH]]))
        s_t1f = sg.tile([K, T * BL], u8, tag="s_t1f")
        nc.sync.dma_start(out=s_t1f[:], in_=tags1f[:])
        s_tnx = sg.tile([128, NCH, K], u8, tag="s_tnx")
        nc.sync.dma_start(out=s_tnx[:],
                          in_=tagsnx[:].rearrange("(n p) k -> p n k", p=128))

        xT = sg.tile([128, 2, T * BL], bf16, tag="xT")
        emit = sg.tile([K, T, BL], f32, tag="emit")
        expE = sg.tile([K, 2, BLK * BL], f32, tag="expE")
        hist = {d: sg.tile([128, 2, T, BL], bf16, tag=f"hist{d}", name=f"hist{d}")
                for d in "fb"}
        hzero = sg.tile([128, 2, BL], bf16, tag="hzero")
        nc.vector.memset(hzero[:], 0.0)

        st_c = {d: sg.tile([128, 2, BL], f32, tag=f"c{d}", name=f"c{d}") for d in "fb"}
        for d in "fb":
            nc.vector.memset(st_c[d][:], 0.0)

        Bv = sg.tile([K, BL], f32, tag="Bv")
        nc.vector.memset(Bv[:], 1.0)
        Eacc = sg.tile([1, BL], f32, tag="Eacc")
        nc.vector.memset(Eacc[:], 0.0)
        Uacc = sg.tile([K, BL], f32, tag="Uacc")
        nc.vector.memset(Uacc[:], 0.0)
        TRbuf = sg.tile([128, NCH], f32, tag="TRbuf")

        # ---- warm-up matmuls ----
        for wt in [s_wih["f"][:, 0, 0:1], s_wih["b"][:, 0, 0:1],
                   s_whh["f"][:, 0, 0:1], s_whh["b"][:, 0, 0:1],
                   s_wout[:, 0, 0:1], ident[:, 0:1]]:
            psd = ps_s.tile([1, 1], f32, tag="pssm", name="psd")
            nc.tensor.matmul(psd[:], lhsT=wt, rhs=wt, start=True, stop=True)
        psd = ps_s.tile([1, 1], f32, tag="pssm", name="psd")
        nc.tensor.matmul(psd[:], lhsT=s_expAT[0:K, 0:1], rhs=s_expAT[0:K, 0:1],
                         start=True, stop=True)

        # ---------------- background work queue ----------------
        bg_q = []

        def drain_bg(n):
            while n > 0 and bg_q:
                bg_q.pop(0)()
                n -= 1

        gathered = [0]

        def gather_chunk(c):
            def work():
                xg = gat.tile([128, E], bf16, tag="xg", name="xg")
                nc.gpsimd.indirect_dma_start(
                    out=xg[:], out_offset=None, in_=emb[:],
                    in_offset=bass.IndirectOffsetOnAxis(ap=idxall[:, c:c + 1], axis=0))
                for k in range(2):
                    pst = ps_t.tile([128, 128], bf16, tag="pst", name="pst")
                    nc.tensor.transpose(out=pst[:], in_=xg[:, k * 128:(k + 1) * 128],
                                        identity=ident[:])
                    nc.vector.tensor_copy(xT[:, k, c * 128:(c + 1) * 128], pst[:])
            return work

        def tag_chunk(i):
            def work():
                tr = gat.tile([128, K], f32, tag="tr", name="tr")
                nc.gpsimd.indirect_dma_start(
                    out=tr[:], out_offset=None, in_=trans[:],
                    in_offset=bass.IndirectOffsetOnAxis(ap=idxtag[:, i:i + 1], axis=0))
                sel = gat.tile([128, K], f32, tag="sel", name="sel")
                nc.vector.tensor_copy(sel[:], s_tnx[:, i, :])
                nc.vector.tensor_tensor(tr[:], tr[:], sel[:], op=OP.mult)
                nc.vector.tensor_reduce(TRbuf[:, i:i + 1], tr[:],
                                        axis=mybir.AxisListType.X, op=OP.add)
            return work

        # ---------------- window build pieces ----------------
        def negdma_piece(t0, ln, nm):
            c0, ncol = t0 * BL, ln * BL

            def work():
                nc.sync.dma_start(out=nm[:, 0:ncol], in_=negm[:, c0:c0 + ncol])
            return work

        def proj_piece(t0, ln, si, d, m, nm):
            c0, ncol = t0 * BL, ln * BL
            sl = slots[si]

            def work():
                o_m = bass.AP(tensor=sl.tensor,
                              offset=sl[:].offset + m * SLOTW * BL,
                              ap=[sl[:].ap[0], [1, ncol]])
                for k in range(2):
                    nc.tensor.matmul(o_m, lhsT=s_wih[d][:, k, m * 128:(m + 1) * 128],
                                     rhs=xT[:, k, c0:c0 + ncol],
                                     start=(m in (0, 4) and k == 0), stop=False,
                                     skip_group_check=True)
                nc.tensor.matmul(o_m, lhsT=s_brow[d][:, m * 128:(m + 1) * 128],
                                 rhs=onesb[:, 0:ncol], start=False, stop=False,
                                 skip_group_check=True)
                if d == "b" and m < 6:
                    nc.tensor.matmul(o_m, lhsT=onesrow[:, 0:128],
                                     rhs=nm[:, 0:ncol], start=False, stop=False,
                                     skip_group_check=True)
            return work

        def build_pieces(j, d):
            t0, ln = (fwin if d == "f" else bwin)[j]
            si = (fslot if d == "f" else bslot)[j]
            pieces = []
            nm = None
            if d == "b":
                nm = neg.tile([1, 256], bf16, tag="nm", name="nm")
                pieces.append(negdma_piece(t0, ln, nm))
            for m in range(8):
                pieces.append(proj_piece(t0, ln, si, d, m, nm))
            return pieces

        # ---------------- LSTM step ----------------
        def lstm_step(d, t):
            j, toff = (fmap if d == "f" else bmap)[t]
            sl = slots[(fslot if d == "f" else bslot)[j]]
            c = st_c[d]
            tprev = t - 1 if d == "f" else t + 1
            hin = hzero if (d == "f" and t == 0) or (d == "b" and t == T - 1) \
                else None
            for m in range(8):
                o_m = bass.AP(tensor=sl.tensor,
                              offset=sl[:].offset + (m * SLOTW + toff) * BL,
                              ap=[sl[:].ap[0], [1, BL]])
                for k in range(2):
                    rhs = hin[:, k, :] if hin is not None else hist[d][:, k, tprev, :]
                    nc.tensor.matmul(o_m, lhsT=s_whh[d][:, k, m * 128:(m + 1) * 128],
                                     rhs=rhs, start=False, stop=False,
                                     skip_group_check=True)
            for g in range(NG):
                cs = slice(g * GB, (g + 1) * GB)
                gin = bass.AP(tensor=sl.tensor,
                              offset=sl[:].offset + toff * BL + g * GB,
                              ap=[sl[:].ap[0], [SLOTW * BL, 8], [1, GB]])
                s = tmp.tile([128, 8, GB], f32, tag=f"s{d}{g}", name=f"s{d}{g}")
                nc.scalar.activation(s[:], gin, AF.Sigmoid)
                si, sf, so, sgt = s[:, 0:2], s[:, 2:4], s[:, 4:6], s[:, 6:8]
                ig = tmp.tile([128, 2, GB], f32, tag=f"ig{d}{g}", name=f"ig{d}{g}")
                nc.vector.scalar_tensor_tensor(ig[:], sgt, 0.5, si,
                                               op0=OP.subtract, op1=OP.mult)
                fc = tmp.tile([128, 2, GB], f32, tag=f"fc{d}{g}", name=f"fc{d}{g}")
                nc.gpsimd.tensor_tensor(fc[:], sf, c[:, :, cs], op=OP.mult)
                nc.vector.scalar_tensor_tensor(c[:, :, cs], ig[:], 2.0, fc[:],
                                               op0=OP.mult, op1=OP.add)
                th = tmp.tile([128, 2, GB], f32, tag=f"th{d}{g}", name=f"th{d}{g}")
                nc.scalar.activation(th[:], c[:, :, cs], AF.Tanh)
                nc.vector.tensor_tensor(hist[d][:, :, t, cs], so, th[:], op=OP.mult)

        # ---------------- emission block assembly ----------------
        def emit_chunk(blk, q):
            """emit[:, blk*32+q*8 : .. +8, :] = woutF@hf + woutB@hb + bias."""
            t0 = blk * BLK + q * 8
            n = 8 * BL

            def work():
                pe = ps_s.tile([K, n], f32, tag="pssm", name="pe")
                for ci, d in ((0, "f"), (2, "b")):
                    for k in range(2):
                        nc.tensor.matmul(
                            pe[:], lhsT=s_wout[:, ci + k, :],
                            rhs=hist[d][:, k, t0:t0 + 8, :].rearrange("p t b -> p (t b)"),
                            start=(ci == 0 and k == 0), stop=(ci == 2 and k == 1))
                nc.scalar.activation(
                    emit[:, t0:t0 + 8, :].rearrange("k t b -> k (t b)"),
                    pe[:], AF.Identity, bias=s_bout[:, 0:1])
            return work

        def unary_piece(blk):
            c0 = blk * BLK * BL
            n = BLK * BL

            def work():
                src = emit[:, blk * BLK:(blk + 1) * BLK, :].rearrange("k t b -> k (t b)")
                t1 = tmp.tile([K, n], f32, tag="t1", name="t1", bufs=1)
                nc.scalar.activation(t1[:], s_t1f[:, c0:c0 + n], AF.Identity)
                um = tmp.tile([K, n], f32, tag="um", name="um", bufs=1)
                nc.gpsimd.tensor_tensor(um[:], t1[:], src, op=OP.mult)
                ur = tmp.tile([K, BL], f32, tag="ur", name="ur")
                umr = bass.AP(tensor=um.tensor, offset=um[:].offset,
                              ap=[um[:].ap[0], [1, BL], [BL, BLK]])
                nc.vector.tensor_reduce(ur[:], umr, axis=mybir.AxisListType.X, op=OP.add)
                nc.vector.tensor_tensor(Uacc[:], Uacc[:], ur[:], op=OP.add)
            return work

        # ---------------- CRF beta tail ----------------
        rescale_count = [0]

        def exp_block(blk):
            src = emit[:, blk * BLK:(blk + 1) * BLK, :].rearrange("k t b -> k (t b)")
            nc.scalar.activation(expE[:, blk % 2, :], src, AF.Exp)

        def _slot_ps(si, col0, parts, ncols):
            sl = slots[si]
            return bass.AP(tensor=sl.tensor, offset=sl[:].offset + col0,
                           ap=[[sl[:].ap[0][0], parts], [1, ncols]])

        def beta_step(s, sub):
            cs = slice(sub * SB, (sub + 1) * SB)
            blk = (s + 1) // BLK
            col = ((s + 1) % BLK) * BL + sub * SB
            bp = tmp.tile([K, SB], f32, tag=f"bp{sub}", name=f"bp{sub}")
            nc.gpsimd.tensor_tensor(bp[:], Bv[:, cs], expE[:, blk % 2, col:col + SB],
                                    op=OP.mult)
            # per-subgroup PSUM bank (window slots are dead in the tail)
            psb = _slot_ps(sub // 2, (sub % 2) * 512, K, SB)
            nc.tensor.matmul(psb, lhsT=s_expAT[:], rhs=bp[:], start=True, stop=True,
                             skip_group_check=True)
            nc.vector.copy_predicated(Bv[:, cs], maskrep[0:K, s + 1, cs], psb)

        def beta_rescale(sub):
            cs = slice(sub * SB, (sub + 1) * SB)
            pss = _slot_ps(2, (sub % 2) * 512 + (sub // 2) * 64, 1, SB)
            nc.tensor.matmul(pss, lhsT=onesf[0:K, 0:1], rhs=Bv[:, cs],
                             start=True, stop=True, skip_group_check=True)
            em = tmp.tile([1, SB], i32, tag=f"em{sub}", name=f"em{sub}")
            nc.vector.tensor_scalar(em[:], pss.bitcast(i32), c_mask[:, 0:1],
                                    None, op0=OP.bitwise_and)
            ef = tmp.tile([1, SB], f32, tag=f"ef{sub}", name=f"ef{sub}")
            nc.vector.tensor_copy(ef[:], em[:])
            nc.vector.scalar_tensor_tensor(Eacc[:, cs], ef[:], 1.0 / (1 << 23),
                                           Eacc[:, cs], op0=OP.mult, op1=OP.add)
            scf = tmp.tile([1, SB], f32, tag=f"scf{sub}", name=f"scf{sub}")
            nc.vector.tensor_scalar(scf[:], ef[:], -1.0, float(0x7F000000),
                                    op0=OP.mult, op1=OP.add)
            sci = tmp.tile([1, SB], i32, tag=f"sci{sub}", name=f"sci{sub}")
            nc.vector.tensor_copy(sci[:], scf[:])
            psr = _slot_ps(2, (sub % 2) * 512 + (sub // 2) * 64 + 16, K, SB)
            nc.tensor.matmul(psr, lhsT=onesf[0:1, 0:K], rhs=sci[:].bitcast(f32),
                             start=True, stop=True, skip_group_check=True)
            nc.vector.tensor_tensor(Bv[:, cs], Bv[:, cs], psr, op=OP.mult)
            rescale_count[0] += 1

        # ================ merged LSTM phase ================
        misc_q = []

        def drain_misc(n):
            while n > 0 and misc_q:
                misc_q.pop(0)()
                n -= 1

        # token gather cursors: fwd consumes chunks ascending, bwd descending
        lo, hi = [0], [NCH - 1]

        def need_lo(upto_col):
            while lo[0] * 128 < upto_col and lo[0] <= hi[0]:
                bg_q.append(gather_chunk(lo[0]))
                lo[0] += 1

        def need_hi(from_col):
            while (hi[0] + 1) * 128 > from_col and hi[0] >= lo[0]:
                bg_q.append(gather_chunk(hi[0]))
                hi[0] -= 1

        # block assembly readiness: blk fully covered at merged step
        # s >= max(blk*BLK+BLK-1, T-1-blk*BLK)
        ready_at = {}
        for blk in range(NBLK):
            ready_at.setdefault(max(blk * BLK + BLK - 1, T - 1 - blk * BLK),
                                []).append(blk)

        # prime: tokens + first windows (fw0, bw0, bw1)
        need_lo(fwin[0][1] * BL)
        need_hi(bwin[1][0] * BL)
        for p in build_pieces(0, "f"):
            bg_q.append(p)
        for p in build_pieces(0, "b"):
            bg_q.append(p)
        for p in build_pieces(1, "b"):
            bg_q.append(p)
        drain_bg(len(bg_q))

        tag_i = [0]
        for s in range(T):
            tf, tb = s, T - 1 - s
            if s >= 4 and s % 4 == 0 and s <= 504:
                k = (s - 4) // 4
                if k % 2 == 0:
                    j = k // 2 + 1
                    need_lo((fwin[j][0] + fwin[j][1]) * BL)
                    for p in build_pieces(j, "f"):
                        bg_q.append(p)
                else:
                    j = (k + 1) // 2 + 1
                    need_hi(bwin[j][0] * BL)
                    for p in build_pieces(j, "b"):
                        bg_q.append(p)
            if s % 8 == 0 and tag_i[0] < NCH:
                misc_q.append(tag_chunk(tag_i[0]))
                tag_i[0] += 1
            lstm_step("f", tf)
            lstm_step("b", tb)
            for blk in ready_at.get(s, []):
                for q in range(4):
                    misc_q.append(emit_chunk(blk, q))
                misc_q.append(unary_piece(blk))
            drain_bg(3)
            drain_misc(1)
        drain_bg(len(bg_q))
        drain_misc(len(misc_q))

        # ================ CRF beta tail ================
        exp_block(NBLK - 1)
        for s in range(T - 2, -1, -1):
            if (s + 1) % BLK == BLK - 1:
                exp_block((s + 1) // BLK)
            for sub in range(NSUB):
                beta_step(s, sub)
            if s % RESCALE == 0 and s > 0:
                for sub in range(NSUB):
                    beta_rescale(sub)

        # ================ finalize ================
        zt = fin.tile([K, BL], f32, tag="zt")
        nc.vector.tensor_tensor(zt[:], Bv[:], expE[:, 0, 0:BL], op=OP.mult)
        psz = ps_s.tile([1, BL], f32, tag="pssm", name="psz")
        nc.tensor.matmul(psz[:], lhsT=onesf[0:K, 0:1], rhs=zt[:], start=True, stop=True)
        logZ = fin.tile([1, BL], f32, tag="logZ")
        nc.scalar.activation(logZ[:], psz[:], AF.Ln)
        nc.vector.scalar_tensor_tensor(logZ[:], Eacc[:], float(np.log(2.0)), logZ[:],
                                       op0=OP.mult, op1=OP.add)
        nc.vector.tensor_scalar(
            logZ[:], logZ[:],
            float(-127.0 * (rescale_count[0] // NSUB) * np.log(2.0)), None,
            op0=OP.add)

        psu = ps_s.tile([1, BL], f32, tag="pssm", name="psu")
        nc.tensor.matmul(psu[:], lhsT=onesf[0:K, 0:1], rhs=Uacc[:], start=True, stop=True)
        score = fin.tile([1, BL], f32, tag="score")
        nc.vector.tensor_copy(score[:], psu[:])

        QT = T // 128
        pstr = ps_s.tile([1, NCH], f32, tag="pssm", name="pstr")
        nc.tensor.matmul(pstr[:], lhsT=onesf[:, 0:1], rhs=TRbuf[:], start=True, stop=True)
        trv = fin.tile([1, BL], f32, tag="trv")
        ptr_ap = bass.AP(tensor=pstr.tensor, offset=pstr[:].offset,
                         ap=[pstr[:].ap[0], [QT, BL], [1, QT]])
        nc.vector.tensor_reduce(trv[:], ptr_ap, axis=mybir.AxisListType.X, op=OP.add)

        dbg = fin.tile([1, 4 * BL], f32, tag="dbg")
        nc.vector.tensor_copy(dbg[:, 0 * BL:1 * BL], score[:])
        nc.vector.tensor_copy(dbg[:, 1 * BL:2 * BL], trv[:])
        nc.vector.tensor_copy(dbg[:, 2 * BL:3 * BL], logZ[:])
        nc.vector.tensor_copy(dbg[:, 3 * BL:4 * BL], Eacc[:])
        nc.sync.dma_start(out=out_dbg[:], in_=dbg[:])

        nc.vector.tensor_tensor(score[:], score[:], trv[:], op=OP.add)
        res = fin.tile([1, BL], f32, tag="res")
        nc.vector.tensor_tensor(res[:], logZ[:], score[:], op=OP.subtract)
        nc.sync.dma_start(out=out_loss[:], in_=res[:])

    nc.compile()
    return nc, names


def _prep_core(inputs, core, perm):
    import ml_dtypes
    bf = ml_dtypes.bfloat16
    s = slice(core * BL, (core + 1) * BL)
    sent = np.asarray(inputs["sentences"][s])
    tags = np.asarray(inputs["tags"][s])
    mask = (sent != PAD_IDX)
    maskT = mask.T
    toks = np.ascontiguousarray(sent.T).reshape(T * BL, 1)
    oh = (tags[:, :, None] == np.arange(K)[None, None, :])
    tags1h = (oh & mask[:, :, None]).transpose(2, 1, 0).reshape(K, T * BL)
    tnx = np.zeros((BL, T, K), np.float32)
    tnx[:, :-1, :] = (oh[:, 1:, :] & mask[:, 1:, None]).astype(np.float32)

    def wprep(wname):
        wt = np.asarray(inputs[wname], np.float32)[perm].copy()
        wt[6 * 128:, :] *= 2.0
        return np.ascontiguousarray(wt.T).astype(bf)

    bvec = {}
    for d, key in (("f", "b_f"), ("b", "b_b")):
        bb = np.asarray(inputs[key], np.float32)[perm].copy()
        bb[6 * 128:] *= 2.0
        bvec[d] = bb.reshape(1, 4 * H).astype(bf)

    return {
        "toks": toks.astype(np.int32),
        "masku": maskT.astype(np.uint8).reshape(1, T * BL),
        "negm": ((~maskT).astype(np.float32) * -1e5).reshape(1, T * BL).astype(bf),
        "tags1f": tags1h.astype(np.uint8),
        "tagsnx": tnx.reshape(T * BL, K).astype(np.uint8),
        "tagsfl": tags.reshape(T * BL, 1).astype(np.int32),
        "emb": np.asarray(inputs["embedding"], np.float32).astype(bf),
        "wih_f": wprep("w_ih_f"), "wih_b": wprep("w_ih_b"),
        "whh_f": wprep("w_hh_f"), "whh_b": wprep("w_hh_b"),
        "brow_f": bvec["f"], "brow_b": bvec["b"],
        "woutT": np.ascontiguousarray(
            np.asarray(inputs["w_out"], np.float32).T.reshape(4, 128, K)).astype(bf),
        "bout": np.asarray(inputs["b_out"]).reshape(K, 1).astype(np.float32),
        "transT": np.ascontiguousarray(np.asarray(inputs["transition"]).T).astype(np.float32),
        "trans": np.asarray(inputs["transition"], np.float32),
    }


def kernel(**inputs):
    from concourse.bass_utils import run_bass_kernel_spmd

    if "prog" not in _cache:
        _cache["prog"] = _build_program()
    nc, names = _cache["prog"]

    blocks = np.arange(4 * H).reshape(4, H)
    perm = np.concatenate([blocks[0], blocks[1], blocks[3], blocks[2]])

    in_maps = []
    for core in range(NCORES):
        m = _prep_core(inputs, core, perm)
        in_maps.append({names[kk]: vv for kk, vv in m.items()})

    res = run_bass_kernel_spmd(nc, in_maps, core_ids=list(range(NCORES)),
                               **_cache.get("run_kwargs", {}))
    out = np.concatenate([r[names["out"]].reshape(BL) for r in res.results])
    _cache["last_results"] = res
    if "dbg" in names:
        _cache["dbg"] = np.concatenate(
            [r[names["dbg"]].reshape(4, BL) for r in res.results], axis=1)
    return out.astype(np.float32)


# revision 25
# speedup vs baseline: 1.1383x; 1.1200x over previous
"""BiLSTM-CRF loss kernel for Trainium2 (8 NeuronCores, data-parallel over batch).

v3: merged-direction LSTM phase + separate pipelined CRF tail.
  - Both LSTM directions run interleaved in ONE loop: two independent
    dependency chains pipeline across PE/Act/DVE/Pool, hiding the per-step
    serial latency that bounded v2.
  - Hidden state written straight into per-direction h histories (bf16);
    emissions are assembled in batched 32-step blocks (4 matmuls + one
    scalar-engine bias pass per 128-col chunk) once both directions have
    covered the block - no per-step emission work at all.
  - Input projection per 4-step window into 1-bank PSUM slots (2 per
    direction, ping-pong); gate bias and the bwd -1e5 pad-kill folded in as
    rank-1 matmuls; recurrence matmuls accumulate in place (start=False).
  - Activations: one sigmoid over all 8 gate chunks per group (g-rows
    pre-doubled; tanh(x)=2*sigmoid(2x)-1 fixed in cell math) + tanh(c) per
    group; sigmoid+tanh share one act table -> no table loads in the loop.
  - Forward direction unmasked (pad suffix garbage is bounded and never
    read); backward masked via the -1e5 gate injection (h=c=0 exactly).
  - CRF beta recursion in exp space as a tail, 2 column-subgroups
    pipelined; exp(emit) batched per block (sigmoid never used in the tail
    -> one act-table load total); rescale every 8 steps via fp32 exponent
    bit extraction (no Ln / reciprocal).
"""

import numpy as np

PAD_IDX = 0
VOCAB, K, E, H = 30000, 20, 256, 256
B, T = 128, 512
NCORES = 8
BL = B // NCORES          # 16 sequences per core
NG = 2                    # batch groups per direction
GB = BL // NG
SLOTW = 8                 # slot capacity (steps); slot = 2 psum banks
BLK = 32                  # emit/exp/unary block size
NBLK = T // BLK
RESCALE = 8               # CRF rescale interval (beta steps, per subgroup)
NSUB = 2                  # CRF column subgroups
SB = BL // NSUB
NCH = 64                  # 128-token gather chunks

_cache = {}


def _build_program():
    from contextlib import ExitStack
    import concourse.bass as bass
    import concourse.bacc as bacc
    import concourse.tile as tile
    from concourse import mybir
    from concourse.masks import make_identity

    f32 = mybir.dt.float32
    i32 = mybir.dt.int32
    bf16 = mybir.dt.bfloat16
    u8 = mybir.dt.uint8
    AF = mybir.ActivationFunctionType
    OP = mybir.AluOpType

    nc = bacc.Bacc(None, target_bir_lowering=False, debug=False)
    names = {}

    with ExitStack() as ctx:
        tc = ctx.enter_context(tile.TileContext(nc))
        dram = ctx.enter_context(tc.tile_pool(name="dram", bufs=1, space="DRAM"))

        def din(key, shape, dt=f32):
            t = dram.tile(shape, dt, kind="ExternalInput", name=key)
            names[key] = t.tensor.name
            return t

        emb = din("emb", [VOCAB, E], bf16)
        toks = din("toks", [T * BL, 1], i32)
        masku = din("masku", [1, T * BL], u8)
        negm = din("negm", [1, T * BL], bf16)
        tags1f = din("tags1f", [K, T * BL], u8)
        tagsnx = din("tagsnx", [T * BL, K], u8)
        tagsfl = din("tagsfl", [T * BL, 1], i32)
        wih = {d: din(f"wih_{d}", [E, 4 * H], bf16) for d in "fb"}
        whh = {d: din(f"whh_{d}", [E, 4 * H], bf16) for d in "fb"}
        brow = {d: din(f"brow_{d}", [1, 4 * H], bf16) for d in "fb"}
        woutT = din("woutT", [4, 128, K], bf16)
        bout = din("bout", [K, 1])
        transT = din("transT", [K, K])
        trans = din("trans", [K, K])
        out_loss = dram.tile([1, BL], f32, kind="ExternalOutput")
        names["out"] = out_loss.tensor.name
        out_dbg = dram.tile([1, 4 * BL], f32, kind="ExternalOutput", name="out_dbg")
        names["dbg"] = out_dbg.tensor.name

        # PSUM: 4 window slots (1 bank each) + transpose + small tiles
        ps_slot = ctx.enter_context(tc.tile_pool(name="ps_slot", bufs=1, space="PSUM"))
        ps_t = ctx.enter_context(tc.tile_pool(name="ps_t", bufs=1, space="PSUM"))
        ps_s = ctx.enter_context(tc.tile_pool(name="ps_s", bufs=1, space="PSUM"))

        sg = ctx.enter_context(tc.tile_pool(name="sg", bufs=1))
        tmp = ctx.enter_context(tc.tile_pool(name="tmp", bufs=4))
        gat = ctx.enter_context(tc.tile_pool(name="gat", bufs=4))
        neg = ctx.enter_context(tc.tile_pool(name="neg", bufs=2))
        fin = ctx.enter_context(tc.tile_pool(name="fin", bufs=3))

        slots = [ps_slot.tile([128, 8, SLOTW, BL], f32, tag=f"slot{i}",
                              name=f"slot{i}") for i in range(3)]

        # window schedule: fwd = 64 x 8-step windows; bwd = 4-step head,
        # 63 x 8-step, 4-step tail -> boundaries stagger every 4 merged steps
        fwin = [(8 * i, 8) for i in range(64)]
        bwin = [(508, 4)] + [(500 - 8 * i, 8) for i in range(63)] + [(0, 4)]
        fslot = [0] * len(fwin)
        bslot = [0] * len(bwin)
        bslot[0], bslot[1] = 1, 2
        rot = [1, 0, 2]
        for k in range(126):
            sl = rot[k % 3]
            if k % 2 == 0:
                fslot[k // 2 + 1] = sl
            else:
                bslot[(k + 1) // 2 + 1] = sl
        # per-t lookup: (window index, toff)
        fmap = [None] * T
        for j, (t0, ln) in enumerate(fwin):
            for o in range(ln):
                fmap[t0 + o] = (j, o)
        bmap = [None] * T
        for j, (t0, ln) in enumerate(bwin):
            for o in range(ln):
                bmap[t0 + o] = (j, o)

        # ---- resident SBUF tensors ----
        s_wih = {d: sg.tile([128, 2, 4 * H], bf16, tag=f"wih{d}", name=f"wih{d}")
                 for d in "fb"}
        s_whh = {d: sg.tile([128, 2, 4 * H], bf16, tag=f"whh{d}", name=f"whh{d}")
                 for d in "fb"}
        s_brow = {d: sg.tile([1, 4 * H], bf16, tag=f"brow{d}", name=f"brow{d}")
                  for d in "fb"}
        for d in "fb":
            nc.sync.dma_start(out=s_wih[d][:], in_=wih[d][:].rearrange("(k p) m -> p k m", p=128))
            nc.sync.dma_start(out=s_whh[d][:], in_=whh[d][:].rearrange("(k p) m -> p k m", p=128))
            nc.sync.dma_start(out=s_brow[d][:], in_=brow[d][:])
        s_wout = sg.tile([128, 4, K], bf16, tag="wout")
        nc.sync.dma_start(out=s_wout[:], in_=woutT[:].rearrange("c p k -> p c k"))
        s_bout = sg.tile([K, 1], f32, tag="bout")
        nc.sync.dma_start(out=s_bout[:], in_=bout[:])
        s_transT = sg.tile([K, K], f32, tag="transT")
        nc.sync.dma_start(out=s_transT[:], in_=transT[:])
        s_expAT = sg.tile([K, K], f32, tag="expAT")
        nc.scalar.activation(s_expAT[:], s_transT[:], AF.Exp)

        onesb = sg.tile([1, 256], bf16, tag="onesb")
        nc.vector.memset(onesb[:], 1.0)
        onesrow = sg.tile([1, 128], bf16, tag="onesrow")
        nc.vector.memset(onesrow[:], 1.0)
        onesf = sg.tile([128, K], f32, tag="onesf")
        nc.vector.memset(onesf[:], 1.0)
        ident = sg.tile([128, 128], bf16, tag="ident")
        make_identity(nc, ident[:])

        c_mask = sg.tile([1, 1], i32, tag="c_mask")
        nc.vector.memset(c_mask[:], 0x7F800000)

        maskrep = sg.tile([128, T, BL], u8, tag="maskrep")
        nc.sync.dma_start(
            out=maskrep[:],
            in_=bass.AP(tensor=masku.tensor, offset=masku[:].offset,
                        ap=[[0, 128], [BL, T], [1, BL]]))

        idxall = sg.tile([128, NCH], i32, tag="idxall")
        nc.sync.dma_start(out=idxall[:],
                          in_=bass.AP(tensor=toks.tensor, offset=toks[:].offset,
                                      ap=[[1, 128], [128, NCH]]))
        idxtag = sg.tile([128, NCH], i32, tag="idxtag")
        nc.sync.dma_start(out=idxtag[:],
                          in_=bass.AP(tensor=tagsfl.tensor, offset=tagsfl[:].offset,
                                      ap=[[1, 128], [128, NCH]]))
        s_t1f = sg.tile([K, T * BL], u8, tag="s_t1f")
        nc.sync.dma_start(out=s_t1f[:], in_=tags1f[:])
        s_tnx = sg.tile([128, NCH, K], u8, tag="s_tnx")
        nc.sync.dma_start(out=s_tnx[:],
                          in_=tagsnx[:].rearrange("(n p) k -> p n k", p=128))

        xT = sg.tile([128, 2, T * BL], bf16, tag="xT")
        emit = sg.tile([K, T, BL], f32, tag="emit")
        expE = sg.tile([K, 2, BLK * BL], f32, tag="expE")
        hist = {d: sg.tile([128, 2, T, BL], bf16, tag=f"hist{d}", name=f"hist{d}")
                for d in "fb"}
        hzero = sg.tile([128, 2, BL], bf16, tag="hzero")
        nc.vector.memset(hzero[:], 0.0)

        st_c = {d: sg.tile([128, 2, BL], f32, tag=f"c{d}", name=f"c{d}") for d in "fb"}
        for d in "fb":
            nc.vector.memset(st_c[d][:], 0.0)

        Bv = sg.tile([K, BL], f32, tag="Bv")
        nc.vector.memset(Bv[:], 1.0)
        Eacc = sg.tile([1, BL], f32, tag="Eacc")
        nc.vector.memset(Eacc[:], 0.0)
        Uacc = sg.tile([K, BL], f32, tag="Uacc")
        nc.vector.memset(Uacc[:], 0.0)
        TRbuf = sg.tile([128, NCH], f32, tag="TRbuf")

        # ---- warm-up matmuls ----
        for wt in [s_wih["f"][:, 0, 0:1], s_wih["b"][:, 0, 0:1],
                   s_whh["f"][:, 0, 0:1], s_whh["b"][:, 0, 0:1],
                   s_wout[:, 0, 0:1], ident[:, 0:1]]:
            psd = ps_s.tile([1, 1], f32, tag="pssm", name="psd")
            nc.tensor.matmul(psd[:], lhsT=wt, rhs=wt, start=True, stop=True)
        psd = ps_s.tile([1, 1], f32, tag="pssm", name="psd")
        nc.tensor.matmul(psd[:], lhsT=s_expAT[0:K, 0:1], rhs=s_expAT[0:K, 0:1],
                         start=True, stop=True)

        # ---------------- background work queue ----------------
        bg_q = []

        def drain_bg(n):
            while n > 0 and bg_q:
                bg_q.pop(0)()
                n -= 1

        gathered = [0]

        def gather_chunk(c):
            def work():
                xg = gat.tile([128, E], bf16, tag="xg", name="xg")
                nc.gpsimd.indirect_dma_start(
                    out=xg[:], out_offset=None, in_=emb[:],
                    in_offset=bass.IndirectOffsetOnAxis(ap=idxall[:, c:c + 1], axis=0))
                for k in range(2):
                    pst = ps_t.tile([128, 128], bf16, tag="pst", name="pst")
                    nc.tensor.transpose(out=pst[:], in_=xg[:, k * 128:(k + 1) * 128],
                                        identity=ident[:])
                    nc.vector.tensor_copy(xT[:, k, c * 128:(c + 1) * 128], pst[:])
            return work

        def tag_chunk(i):
            def work():
                tr = gat.tile([128, K], f32, tag="tr", name="tr")
                nc.gpsimd.indirect_dma_start(
                    out=tr[:], out_offset=None, in_=trans[:],
                    in_offset=bass.IndirectOffsetOnAxis(ap=idxtag[:, i:i + 1], axis=0))
                sel = gat.tile([128, K], f32, tag="sel", name="sel")
                nc.vector.tensor_copy(sel[:], s_tnx[:, i, :])
                nc.vector.tensor_tensor(tr[:], tr[:], sel[:], op=OP.mult)
                nc.vector.tensor_reduce(TRbuf[:, i:i + 1], tr[:],
                                        axis=mybir.AxisListType.X, op=OP.add)
            return work

        # ---------------- window build pieces ----------------
        def negdma_piece(t0, ln, nm):
            c0, ncol = t0 * BL, ln * BL

            def work():
                nc.sync.dma_start(out=nm[:, 0:ncol], in_=negm[:, c0:c0 + ncol])
            return work

        def proj_piece(t0, ln, si, d, m, nm):
            c0, ncol = t0 * BL, ln * BL
            sl = slots[si]

            def work():
                o_m = bass.AP(tensor=sl.tensor,
                              offset=sl[:].offset + m * SLOTW * BL,
                              ap=[sl[:].ap[0], [1, ncol]])
                for k in range(2):
                    nc.tensor.matmul(o_m, lhsT=s_wih[d][:, k, m * 128:(m + 1) * 128],
                                     rhs=xT[:, k, c0:c0 + ncol],
                                     start=(m in (0, 4) and k == 0), stop=False,
                                     skip_group_check=True)
                nc.tensor.matmul(o_m, lhsT=s_brow[d][:, m * 128:(m + 1) * 128],
                                 rhs=onesb[:, 0:ncol], start=False, stop=False,
                                 skip_group_check=True)
                if d == "b" and m < 6:
                    nc.tensor.matmul(o_m, lhsT=onesrow[:, 0:128],
                                     rhs=nm[:, 0:ncol], start=False, stop=False,
                                     skip_group_check=True)
            return work

        def build_pieces(j, d):
            t0, ln = (fwin if d == "f" else bwin)[j]
            si = (fslot if d == "f" else bslot)[j]
            pieces = []
            nm = None
            if d == "b":
                nm = neg.tile([1, 256], bf16, tag="nm", name="nm")
                pieces.append(negdma_piece(t0, ln, nm))
            for m in range(8):
                pieces.append(proj_piece(t0, ln, si, d, m, nm))
            return pieces

        # ---------------- LSTM step ----------------
        def lstm_step(d, t):
            j, toff = (fmap if d == "f" else bmap)[t]
            sl = slots[(fslot if d == "f" else bslot)[j]]
            c = st_c[d]
            tprev = t - 1 if d == "f" else t + 1
            hin = hzero if (d == "f" and t == 0) or (d == "b" and t == T - 1) \
                else None
            for m in range(8):
                o_m = bass.AP(tensor=sl.tensor,
                              offset=sl[:].offset + (m * SLOTW + toff) * BL,
                              ap=[sl[:].ap[0], [1, BL]])
                for k in range(2):
                    rhs = hin[:, k, :] if hin is not None else hist[d][:, k, tprev, :]
                    nc.tensor.matmul(o_m, lhsT=s_whh[d][:, k, m * 128:(m + 1) * 128],
                                     rhs=rhs, start=False, stop=False,
                                     skip_group_check=True)
            gin = bass.AP(tensor=sl.tensor,
                          offset=sl[:].offset + toff * BL,
                          ap=[sl[:].ap[0], [SLOTW * BL, 8], [1, BL]])
            s = tmp.tile([128, 8, BL], f32, tag=f"s{d}", name=f"s{d}")
            nc.scalar.activation(s[:], gin, AF.Sigmoid)
            for g in range(NG):
                cs = slice(g * GB, (g + 1) * GB)
                si, sf, so, sgt = (s[:, 0:2, cs], s[:, 2:4, cs],
                                   s[:, 4:6, cs], s[:, 6:8, cs])
                ig = tmp.tile([128, 2, GB], f32, tag=f"ig{d}{g}", name=f"ig{d}{g}")
                nc.vector.scalar_tensor_tensor(ig[:], sgt, 0.5, si,
                                               op0=OP.subtract, op1=OP.mult)
                fc = tmp.tile([128, 2, GB], f32, tag=f"fc{d}{g}", name=f"fc{d}{g}")
                nc.gpsimd.tensor_tensor(fc[:], sf, c[:, :, cs], op=OP.mult)
                nc.vector.scalar_tensor_tensor(c[:, :, cs], ig[:], 2.0, fc[:],
                                               op0=OP.mult, op1=OP.add)
                th = tmp.tile([128, 2, GB], f32, tag=f"th{d}{g}", name=f"th{d}{g}")
                nc.scalar.activation(th[:], c[:, :, cs], AF.Tanh)
                nc.vector.tensor_tensor(hist[d][:, :, t, cs], so, th[:], op=OP.mult)

        # ---------------- emission block assembly ----------------
        def emit_chunk(blk, q):
            """emit[:, blk*32+q*8 : .. +8, :] = woutF@hf + woutB@hb + bias."""
            t0 = blk * BLK + q * 8
            n = 8 * BL

            def work():
                pe = ps_s.tile([K, n], f32, tag="pssm", name="pe")
                for ci, d in ((0, "f"), (2, "b")):
                    for k in range(2):
                        nc.tensor.matmul(
                            pe[:], lhsT=s_wout[:, ci + k, :],
                            rhs=hist[d][:, k, t0:t0 + 8, :].rearrange("p t b -> p (t b)"),
                            start=(ci == 0 and k == 0), stop=(ci == 2 and k == 1))
                nc.scalar.activation(
                    emit[:, t0:t0 + 8, :].rearrange("k t b -> k (t b)"),
                    pe[:], AF.Identity, bias=s_bout[:, 0:1])
            return work

        def unary_piece(blk):
            c0 = blk * BLK * BL
            n = BLK * BL

            def work():
                src = emit[:, blk * BLK:(blk + 1) * BLK, :].rearrange("k t b -> k (t b)")
                t1 = tmp.tile([K, n], f32, tag="t1", name="t1", bufs=1)
                nc.scalar.activation(t1[:], s_t1f[:, c0:c0 + n], AF.Identity)
                um = tmp.tile([K, n], f32, tag="um", name="um", bufs=1)
                nc.gpsimd.tensor_tensor(um[:], t1[:], src, op=OP.mult)
                ur = tmp.tile([K, BL], f32, tag="ur", name="ur")
                umr = bass.AP(tensor=um.tensor, offset=um[:].offset,
                              ap=[um[:].ap[0], [1, BL], [BL, BLK]])
                nc.vector.tensor_reduce(ur[:], umr, axis=mybir.AxisListType.X, op=OP.add)
                nc.vector.tensor_tensor(Uacc[:], Uacc[:], ur[:], op=OP.add)
            return work

        # ---------------- CRF beta tail ----------------
        rescale_count = [0]

        def exp_block(blk):
            src = emit[:, blk * BLK:(blk + 1) * BLK, :].rearrange("k t b -> k (t b)")
            nc.scalar.activation(expE[:, blk % 2, :], src, AF.Exp)

        def _slot_ps(si, col0, parts, ncols):
            sl = slots[si]
            return bass.AP(tensor=sl.tensor, offset=sl[:].offset + col0,
                           ap=[[sl[:].ap[0][0], parts], [1, ncols]])

        def beta_step(s, sub):
            cs = slice(sub * SB, (sub + 1) * SB)
            blk = (s + 1) // BLK
            col = ((s + 1) % BLK) * BL + sub * SB
            bp = tmp.tile([K, SB], f32, tag=f"bp{sub}", name=f"bp{sub}")
            nc.gpsimd.tensor_tensor(bp[:], Bv[:, cs], expE[:, blk % 2, col:col + SB],
                                    op=OP.mult)
            # per-subgroup PSUM bank (window slots are dead in the tail)
            psb = _slot_ps(sub // 2, (sub % 2) * 512, K, SB)
            nc.tensor.matmul(psb, lhsT=s_expAT[:], rhs=bp[:], start=True, stop=True,
                             skip_group_check=True)
            nc.vector.copy_predicated(Bv[:, cs], maskrep[0:K, s + 1, cs], psb)

        def beta_rescale(sub):
            cs = slice(sub * SB, (sub + 1) * SB)
            pss = _slot_ps(2, (sub % 2) * 512 + (sub // 2) * 64, 1, SB)
            nc.tensor.matmul(pss, lhsT=onesf[0:K, 0:1], rhs=Bv[:, cs],
                             start=True, stop=True, skip_group_check=True)
            em = tmp.tile([1, SB], i32, tag=f"em{sub}", name=f"em{sub}")
            nc.vector.tensor_scalar(em[:], pss.bitcast(i32), c_mask[:, 0:1],
                                    None, op0=OP.bitwise_and)
            ef = tmp.tile([1, SB], f32, tag=f"ef{sub}", name=f"ef{sub}")
            nc.vector.tensor_copy(ef[:], em[:])
            nc.vector.scalar_tensor_tensor(Eacc[:, cs], ef[:], 1.0 / (1 << 23),
                                           Eacc[:, cs], op0=OP.mult, op1=OP.add)
            scf = tmp.tile([1, SB], f32, tag=f"scf{sub}", name=f"scf{sub}")
            nc.vector.tensor_scalar(scf[:], ef[:], -1.0, float(0x7F000000),
                                    op0=OP.mult, op1=OP.add)
            sci = tmp.tile([1, SB], i32, tag=f"sci{sub}", name=f"sci{sub}")
            nc.vector.tensor_copy(sci[:], scf[:])
            psr = _slot_ps(2, (sub % 2) * 512 + (sub // 2) * 64 + 16, K, SB)
            nc.tensor.matmul(psr, lhsT=onesf[0:1, 0:K], rhs=sci[:].bitcast(f32),
                             start=True, stop=True, skip_group_check=True)
            nc.vector.tensor_tensor(Bv[:, cs], Bv[:, cs], psr, op=OP.mult)
            rescale_count[0] += 1

        # ================ merged LSTM phase ================
        misc_q = []

        def drain_misc(n):
            while n > 0 and misc_q:
                misc_q.pop(0)()
                n -= 1

        # token gather cursors: fwd consumes chunks ascending, bwd descending
        lo, hi = [0], [NCH - 1]

        def need_lo(upto_col):
            while lo[0] * 128 < upto_col and lo[0] <= hi[0]:
                bg_q.append(gather_chunk(lo[0]))
                lo[0] += 1

        def need_hi(from_col):
            while (hi[0] + 1) * 128 > from_col and hi[0] >= lo[0]:
                bg_q.append(gather_chunk(hi[0]))
                hi[0] -= 1

        # block assembly readiness: blk fully covered at merged step
        # s >= max(blk*BLK+BLK-1, T-1-blk*BLK)
        ready_at = {}
        for blk in range(NBLK):
            ready_at.setdefault(max(blk * BLK + BLK - 1, T - 1 - blk * BLK),
                                []).append(blk)

        # prime: tokens + first windows (fw0, bw0, bw1)
        need_lo(fwin[0][1] * BL)
        need_hi(bwin[1][0] * BL)
        for p in build_pieces(0, "f"):
            bg_q.append(p)
        for p in build_pieces(0, "b"):
            bg_q.append(p)
        for p in build_pieces(1, "b"):
            bg_q.append(p)
        drain_bg(len(bg_q))

        tag_i = [0]
        for s in range(T):
            tf, tb = s, T - 1 - s
            if s >= 4 and s % 4 == 0 and s <= 504:
                k = (s - 4) // 4
                if k % 2 == 0:
                    j = k // 2 + 1
                    need_lo((fwin[j][0] + fwin[j][1]) * BL)
                    for p in build_pieces(j, "f"):
                        bg_q.append(p)
                else:
                    j = (k + 1) // 2 + 1
                    need_hi(bwin[j][0] * BL)
                    for p in build_pieces(j, "b"):
                        bg_q.append(p)
            if s % 8 == 0 and tag_i[0] < NCH:
                misc_q.append(tag_chunk(tag_i[0]))
                tag_i[0] += 1
            lstm_step("f", tf)
            lstm_step("b", tb)
            for blk in ready_at.get(s, []):
                for q in range(4):
                    misc_q.append(emit_chunk(blk, q))
                misc_q.append(unary_piece(blk))
            drain_bg(3)
            drain_misc(1)
        drain_bg(len(bg_q))
        drain_misc(len(misc_q))

        # ================ CRF beta tail ================
        exp_block(NBLK - 1)
        for s in range(T - 2, -1, -1):
            if (s + 1) % BLK == BLK - 1:
                exp_block((s + 1) // BLK)
            for sub in range(NSUB):
                beta_step(s, sub)
            if s % RESCALE == 0 and s > 0:
                for sub in range(NSUB):
                    beta_rescale(sub)

        # ================ finalize ================
        zt = fin.tile([K, BL], f32, tag="zt")
        nc.vector.tensor_tensor(zt[:], Bv[:], expE[:, 0, 0:BL], op=OP.mult)
        psz = ps_s.tile([1, BL], f32, tag="pssm", name="psz")
        nc.tensor.matmul(psz[:], lhsT=onesf[0:K, 0:1], rhs=zt[:], start=True, stop=True)
        logZ = fin.tile([1, BL], f32, tag="logZ")
        nc.scalar.activation(logZ[:], psz[:], AF.Ln)
        nc.vector.scalar_tensor_tensor(logZ[:], Eacc[:], float(np.log(2.0)), logZ[:],
                                       op0=OP.mult, op1=OP.add)
        nc.vector.tensor_scalar(
            logZ[:], logZ[:],
            float(-127.0 * (rescale_count[0] // NSUB) * np.log(2.0)), None,
            op0=OP.add)

        psu = ps_s.tile([1, BL], f32, tag="pssm", name="psu")
        nc.tensor.matmul(psu[:], lhsT=onesf[0:K, 0:1], rhs=Uacc[:], start=True, stop=True)
        score = fin.tile([1, BL], f32, tag="score")
        nc.vector.tensor_copy(score[:], psu[:])

        QT = T // 128
        pstr = ps_s.tile([1, NCH], f32, tag="pssm", name="pstr")
        nc.tensor.matmul(pstr[:], lhsT=onesf[:, 0:1], rhs=TRbuf[:], start=True, stop=True)
        trv = fin.tile([1, BL], f32, tag="trv")
        ptr_ap = bass.AP(tensor=pstr.tensor, offset=pstr[:].offset,
                         ap=[pstr[:].ap[0], [QT, BL], [1, QT]])
        nc.vector.tensor_reduce(trv[:], ptr_ap, axis=mybir.AxisListType.X, op=OP.add)

        dbg = fin.tile([1, 4 * BL], f32, tag="dbg")
        nc.vector.tensor_copy(dbg[:, 0 * BL:1 * BL], score[:])
        nc.vector.tensor_copy(dbg[:, 1 * BL:2 * BL], trv[:])
        nc.vector.tensor_copy(dbg[:, 2 * BL:3 * BL], logZ[:])
        nc.vector.tensor_copy(dbg[:, 3 * BL:4 * BL], Eacc[:])
        nc.sync.dma_start(out=out_dbg[:], in_=dbg[:])

        nc.vector.tensor_tensor(score[:], score[:], trv[:], op=OP.add)
        res = fin.tile([1, BL], f32, tag="res")
        nc.vector.tensor_tensor(res[:], logZ[:], score[:], op=OP.subtract)
        nc.sync.dma_start(out=out_loss[:], in_=res[:])

    nc.compile()
    return nc, names


def _prep_core(inputs, core, perm):
    import ml_dtypes
    bf = ml_dtypes.bfloat16
    s = slice(core * BL, (core + 1) * BL)
    sent = np.asarray(inputs["sentences"][s])
    tags = np.asarray(inputs["tags"][s])
    mask = (sent != PAD_IDX)
    maskT = mask.T
    toks = np.ascontiguousarray(sent.T).reshape(T * BL, 1)
    oh = (tags[:, :, None] == np.arange(K)[None, None, :])
    tags1h = (oh & mask[:, :, None]).transpose(2, 1, 0).reshape(K, T * BL)
    tnx = np.zeros((BL, T, K), np.float32)
    tnx[:, :-1, :] = (oh[:, 1:, :] & mask[:, 1:, None]).astype(np.float32)

    def wprep(wname):
        wt = np.asarray(inputs[wname], np.float32)[perm].copy()
        wt[6 * 128:, :] *= 2.0
        return np.ascontiguousarray(wt.T).astype(bf)

    bvec = {}
    for d, key in (("f", "b_f"), ("b", "b_b")):
        bb = np.asarray(inputs[key], np.float32)[perm].copy()
        bb[6 * 128:] *= 2.0
        bvec[d] = bb.reshape(1, 4 * H).astype(bf)

    return {
        "toks": toks.astype(np.int32),
        "masku": maskT.astype(np.uint8).reshape(1, T * BL),
        "negm": ((~maskT).astype(np.float32) * -1e5).reshape(1, T * BL).astype(bf),
        "tags1f": tags1h.astype(np.uint8),
        "tagsnx": tnx.reshape(T * BL, K).astype(np.uint8),
        "tagsfl": tags.reshape(T * BL, 1).astype(np.int32),
        "emb": np.asarray(inputs["embedding"], np.float32).astype(bf),
        "wih_f": wprep("w_ih_f"), "wih_b": wprep("w_ih_b"),
        "whh_f": wprep("w_hh_f"), "whh_b": wprep("w_hh_b"),
        "brow_f": bvec["f"], "brow_b": bvec["b"],
        "woutT": np.ascontiguousarray(
            np.asarray(inputs["w_out"], np.float32).T.reshape(4, 128, K)).astype(bf),
        "bout": np.asarray(inputs["b_out"]).reshape(K, 1).astype(np.float32),
        "transT": np.ascontiguousarray(np.asarray(inputs["transition"]).T).astype(np.float32),
        "trans": np.asarray(inputs["transition"], np.float32),
    }


def kernel(**inputs):
    from concourse.bass_utils import run_bass_kernel_spmd

    if "prog" not in _cache:
        _cache["prog"] = _build_program()
    nc, names = _cache["prog"]

    blocks = np.arange(4 * H).reshape(4, H)
    perm = np.concatenate([blocks[0], blocks[1], blocks[3], blocks[2]])

    in_maps = []
    for core in range(NCORES):
        m = _prep_core(inputs, core, perm)
        in_maps.append({names[kk]: vv for kk, vv in m.items()})

    res = run_bass_kernel_spmd(nc, in_maps, core_ids=list(range(NCORES)),
                               **_cache.get("run_kwargs", {}))
    out = np.concatenate([r[names["out"]].reshape(BL) for r in res.results])
    _cache["last_results"] = res
    if "dbg" in names:
        _cache["dbg"] = np.concatenate(
            [r[names["dbg"]].reshape(4, BL) for r in res.results], axis=1)
    return out.astype(np.float32)


# revision 26
# speedup vs baseline: 1.1506x; 1.0108x over previous
"""BiLSTM-CRF loss kernel for Trainium2 (8 NeuronCores, data-parallel over batch).

v3: merged-direction LSTM phase + separate pipelined CRF tail.
  - Both LSTM directions run interleaved in ONE loop: two independent
    dependency chains pipeline across PE/Act/DVE/Pool, hiding the per-step
    serial latency that bounded v2.
  - Hidden state written straight into per-direction h histories (bf16);
    emissions are assembled in batched 32-step blocks (4 matmuls + one
    scalar-engine bias pass per 128-col chunk) once both directions have
    covered the block - no per-step emission work at all.
  - Input projection per 4-step window into 1-bank PSUM slots (2 per
    direction, ping-pong); gate bias and the bwd -1e5 pad-kill folded in as
    rank-1 matmuls; recurrence matmuls accumulate in place (start=False).
  - Activations: one sigmoid over all 8 gate chunks per group (g-rows
    pre-doubled; tanh(x)=2*sigmoid(2x)-1 fixed in cell math) + tanh(c) per
    group; sigmoid+tanh share one act table -> no table loads in the loop.
  - Forward direction unmasked (pad suffix garbage is bounded and never
    read); backward masked via the -1e5 gate injection (h=c=0 exactly).
  - CRF beta recursion in exp space as a tail, 2 column-subgroups
    pipelined; exp(emit) batched per block (sigmoid never used in the tail
    -> one act-table load total); rescale every 8 steps via fp32 exponent
    bit extraction (no Ln / reciprocal).
"""

import numpy as np

PAD_IDX = 0
VOCAB, K, E, H = 30000, 20, 256, 256
B, T = 128, 512
NCORES = 8
BL = B // NCORES          # 16 sequences per core
NG = 2                    # batch groups per direction
GB = BL // NG
SLOTW = 8                 # slot capacity (steps); slot = 2 psum banks
BLK = 32                  # emit/exp/unary block size
NBLK = T // BLK
RESCALE = 8               # CRF rescale interval (beta steps, per subgroup)
NSUB = 2                  # CRF column subgroups
SB = BL // NSUB
NCH = 64                  # 128-token gather chunks

_cache = {}


def _build_program():
    from contextlib import ExitStack
    import concourse.bass as bass
    import concourse.bacc as bacc
    import concourse.tile as tile
    from concourse import mybir
    from concourse.masks import make_identity

    f32 = mybir.dt.float32
    i32 = mybir.dt.int32
    bf16 = mybir.dt.bfloat16
    u8 = mybir.dt.uint8
    AF = mybir.ActivationFunctionType
    OP = mybir.AluOpType

    nc = bacc.Bacc(None, target_bir_lowering=False, debug=False)
    names = {}

    with ExitStack() as ctx:
        tc = ctx.enter_context(tile.TileContext(nc))
        dram = ctx.enter_context(tc.tile_pool(name="dram", bufs=1, space="DRAM"))

        def din(key, shape, dt=f32):
            t = dram.tile(shape, dt, kind="ExternalInput", name=key)
            names[key] = t.tensor.name
            return t

        emb = din("emb", [VOCAB, E], bf16)
        toks = din("toks", [T * BL, 1], i32)
        masku = din("masku", [1, T * BL], u8)
        negm = din("negm", [1, T * BL], bf16)
        tags1f = din("tags1f", [K, T * BL], u8)
        tagsnx = din("tagsnx", [T * BL, K], u8)
        tagsfl = din("tagsfl", [T * BL, 1], i32)
        wih = {d: din(f"wih_{d}", [E, 4 * H], bf16) for d in "fb"}
        whh = {d: din(f"whh_{d}", [E, 4 * H], bf16) for d in "fb"}
        brow = {d: din(f"brow_{d}", [1, 4 * H], bf16) for d in "fb"}
        woutT = din("woutT", [4, 128, K], bf16)
        bout = din("bout", [K, 1])
        transT = din("transT", [K, K])
        trans = din("trans", [K, K])
        out_loss = dram.tile([1, BL], f32, kind="ExternalOutput")
        names["out"] = out_loss.tensor.name
        out_dbg = dram.tile([1, 4 * BL], f32, kind="ExternalOutput", name="out_dbg")
        names["dbg"] = out_dbg.tensor.name

        # PSUM: 4 window slots (1 bank each) + transpose + small tiles
        ps_slot = ctx.enter_context(tc.tile_pool(name="ps_slot", bufs=1, space="PSUM"))
        ps_t = ctx.enter_context(tc.tile_pool(name="ps_t", bufs=1, space="PSUM"))
        ps_s = ctx.enter_context(tc.tile_pool(name="ps_s", bufs=1, space="PSUM"))

        sg = ctx.enter_context(tc.tile_pool(name="sg", bufs=1))
        tmp = ctx.enter_context(tc.tile_pool(name="tmp", bufs=4))
        gat = ctx.enter_context(tc.tile_pool(name="gat", bufs=4))
        neg = ctx.enter_context(tc.tile_pool(name="neg", bufs=2))
        fin = ctx.enter_context(tc.tile_pool(name="fin", bufs=3))

        slots = [ps_slot.tile([128, 8, SLOTW, BL], f32, tag=f"slot{i}",
                              name=f"slot{i}") for i in range(3)]

        # window schedule: fwd = 64 x 8-step windows; bwd = 4-step head,
        # 63 x 8-step, 4-step tail -> boundaries stagger every 4 merged steps
        fwin = [(8 * i, 8) for i in range(64)]
        bwin = [(508, 4)] + [(500 - 8 * i, 8) for i in range(63)] + [(0, 4)]
        fslot = [0] * len(fwin)
        bslot = [0] * len(bwin)
        bslot[0], bslot[1] = 1, 2
        rot = [1, 0, 2]
        for k in range(126):
            sl = rot[k % 3]
            if k % 2 == 0:
                fslot[k // 2 + 1] = sl
            else:
                bslot[(k + 1) // 2 + 1] = sl
        # per-t lookup: (window index, toff)
        fmap = [None] * T
        for j, (t0, ln) in enumerate(fwin):
            for o in range(ln):
                fmap[t0 + o] = (j, o)
        bmap = [None] * T
        for j, (t0, ln) in enumerate(bwin):
            for o in range(ln):
                bmap[t0 + o] = (j, o)

        # ---- resident SBUF tensors ----
        s_wih = {d: sg.tile([128, 2, 4 * H], bf16, tag=f"wih{d}", name=f"wih{d}")
                 for d in "fb"}
        s_whh = {d: sg.tile([128, 2, 4 * H], bf16, tag=f"whh{d}", name=f"whh{d}")
                 for d in "fb"}
        s_brow = {d: sg.tile([1, 4 * H], bf16, tag=f"brow{d}", name=f"brow{d}")
                  for d in "fb"}
        for d in "fb":
            nc.sync.dma_start(out=s_wih[d][:], in_=wih[d][:].rearrange("(k p) m -> p k m", p=128))
            nc.sync.dma_start(out=s_whh[d][:], in_=whh[d][:].rearrange("(k p) m -> p k m", p=128))
            nc.sync.dma_start(out=s_brow[d][:], in_=brow[d][:])
        s_wout = sg.tile([128, 4, K], bf16, tag="wout")
        nc.sync.dma_start(out=s_wout[:], in_=woutT[:].rearrange("c p k -> p c k"))
        s_bout = sg.tile([K, 1], f32, tag="bout")
        nc.sync.dma_start(out=s_bout[:], in_=bout[:])
        s_transT = sg.tile([K, K], f32, tag="transT")
        nc.sync.dma_start(out=s_transT[:], in_=transT[:])
        s_expAT = sg.tile([K, K], f32, tag="expAT")
        nc.scalar.activation(s_expAT[:], s_transT[:], AF.Exp)

        onesb = sg.tile([1, 256], bf16, tag="onesb")
        nc.vector.memset(onesb[:], 1.0)
        onesrow = sg.tile([1, 128], bf16, tag="onesrow")
        nc.vector.memset(onesrow[:], 1.0)
        onesf = sg.tile([128, K], f32, tag="onesf")
        nc.vector.memset(onesf[:], 1.0)
        ident = sg.tile([128, 128], bf16, tag="ident")
        make_identity(nc, ident[:])

        c_mask = sg.tile([1, 1], i32, tag="c_mask")
        nc.vector.memset(c_mask[:], 0x7F800000)

        maskrep = sg.tile([128, T, BL], u8, tag="maskrep")
        nc.sync.dma_start(
            out=maskrep[:],
            in_=bass.AP(tensor=masku.tensor, offset=masku[:].offset,
                        ap=[[0, 128], [BL, T], [1, BL]]))

        idxall = sg.tile([128, NCH], i32, tag="idxall")
        nc.sync.dma_start(out=idxall[:],
                          in_=bass.AP(tensor=toks.tensor, offset=toks[:].offset,
                                      ap=[[1, 128], [128, NCH]]))
        idxtag = sg.tile([128, NCH], i32, tag="idxtag")
        nc.sync.dma_start(out=idxtag[:],
                          in_=bass.AP(tensor=tagsfl.tensor, offset=tagsfl[:].offset,
                                      ap=[[1, 128], [128, NCH]]))
        s_t1f = sg.tile([K, T * BL], u8, tag="s_t1f")
        nc.sync.dma_start(out=s_t1f[:], in_=tags1f[:])
        s_tnx = sg.tile([128, NCH, K], u8, tag="s_tnx")
        nc.sync.dma_start(out=s_tnx[:],
                          in_=tagsnx[:].rearrange("(n p) k -> p n k", p=128))

        xT = sg.tile([128, 2, T * BL], bf16, tag="xT")
        emit = sg.tile([K, T, BL], f32, tag="emit")
        expE = sg.tile([K, 2, BLK * BL], f32, tag="expE")
        hist = {d: sg.tile([128, 2, T, BL], bf16, tag=f"hist{d}", name=f"hist{d}")
                for d in "fb"}
        hzero = sg.tile([128, 2, BL], bf16, tag="hzero")
        nc.vector.memset(hzero[:], 0.0)

        st_c = {d: sg.tile([128, 2, BL], f32, tag=f"c{d}", name=f"c{d}") for d in "fb"}
        for d in "fb":
            nc.vector.memset(st_c[d][:], 0.0)

        Bv = sg.tile([K, BL], f32, tag="Bv")
        nc.vector.memset(Bv[:], 1.0)
        Eacc = sg.tile([1, BL], f32, tag="Eacc")
        nc.vector.memset(Eacc[:], 0.0)
        Uacc = sg.tile([K, BL], f32, tag="Uacc")
        nc.vector.memset(Uacc[:], 0.0)
        TRbuf = sg.tile([128, NCH], f32, tag="TRbuf")

        # ---- warm-up matmuls ----
        for wt in [s_wih["f"][:, 0, 0:1], s_wih["b"][:, 0, 0:1],
                   s_whh["f"][:, 0, 0:1], s_whh["b"][:, 0, 0:1],
                   s_wout[:, 0, 0:1], ident[:, 0:1]]:
            psd = ps_s.tile([1, 1], f32, tag="pssm", name="psd")
            nc.tensor.matmul(psd[:], lhsT=wt, rhs=wt, start=True, stop=True)
        psd = ps_s.tile([1, 1], f32, tag="pssm", name="psd")
        nc.tensor.matmul(psd[:], lhsT=s_expAT[0:K, 0:1], rhs=s_expAT[0:K, 0:1],
                         start=True, stop=True)

        # ---------------- background work queue ----------------
        bg_q = []

        def drain_bg(n):
            while n > 0 and bg_q:
                bg_q.pop(0)()
                n -= 1

        gathered = [0]

        def gather_chunk(c):
            def work():
                xg = gat.tile([128, E], bf16, tag="xg", name="xg")
                nc.gpsimd.indirect_dma_start(
                    out=xg[:], out_offset=None, in_=emb[:],
                    in_offset=bass.IndirectOffsetOnAxis(ap=idxall[:, c:c + 1], axis=0))
                for k in range(2):
                    pst = ps_t.tile([128, 128], bf16, tag="pst", name="pst")
                    nc.tensor.transpose(out=pst[:], in_=xg[:, k * 128:(k + 1) * 128],
                                        identity=ident[:])
                    nc.vector.tensor_copy(xT[:, k, c * 128:(c + 1) * 128], pst[:])
            return work

        def tag_chunk(i):
            def work():
                tr = gat.tile([128, K], f32, tag="tr", name="tr")
                nc.gpsimd.indirect_dma_start(
                    out=tr[:], out_offset=None, in_=trans[:],
                    in_offset=bass.IndirectOffsetOnAxis(ap=idxtag[:, i:i + 1], axis=0))
                sel = gat.tile([128, K], f32, tag="sel", name="sel")
                nc.vector.tensor_copy(sel[:], s_tnx[:, i, :])
                nc.vector.tensor_tensor(tr[:], tr[:], sel[:], op=OP.mult)
                nc.vector.tensor_reduce(TRbuf[:, i:i + 1], tr[:],
                                        axis=mybir.AxisListType.X, op=OP.add)
            return work

        # ---------------- window build pieces ----------------
        def negdma_piece(t0, ln, nm):
            c0, ncol = t0 * BL, ln * BL

            def work():
                nc.sync.dma_start(out=nm[:, 0:ncol], in_=negm[:, c0:c0 + ncol])
            return work

        def proj_piece(t0, ln, si, d, m, nm):
            c0, ncol = t0 * BL, ln * BL
            sl = slots[si]

            def work():
                o_m = bass.AP(tensor=sl.tensor,
                              offset=sl[:].offset + m * SLOTW * BL,
                              ap=[sl[:].ap[0], [1, ncol]])
                for k in range(2):
                    nc.tensor.matmul(o_m, lhsT=s_wih[d][:, k, m * 128:(m + 1) * 128],
                                     rhs=xT[:, k, c0:c0 + ncol],
                                     start=(m in (0, 4) and k == 0), stop=False,
                                     skip_group_check=True)
                nc.tensor.matmul(o_m, lhsT=s_brow[d][:, m * 128:(m + 1) * 128],
                                 rhs=onesb[:, 0:ncol], start=False, stop=False,
                                 skip_group_check=True)
                if d == "b" and m < 6:
                    nc.tensor.matmul(o_m, lhsT=onesrow[:, 0:128],
                                     rhs=nm[:, 0:ncol], start=False, stop=False,
                                     skip_group_check=True)
            return work

        def build_pieces(j, d):
            t0, ln = (fwin if d == "f" else bwin)[j]
            si = (fslot if d == "f" else bslot)[j]
            pieces = []
            nm = None
            if d == "b":
                nm = neg.tile([1, 256], bf16, tag="nm", name="nm")
                pieces.append(negdma_piece(t0, ln, nm))
            for m in range(8):
                pieces.append(proj_piece(t0, ln, si, d, m, nm))
            return pieces

        # ---------------- LSTM step ----------------
        def lstm_step(d, t):
            j, toff = (fmap if d == "f" else bmap)[t]
            sl = slots[(fslot if d == "f" else bslot)[j]]
            c = st_c[d]
            tprev = t - 1 if d == "f" else t + 1
            hin = hzero if (d == "f" and t == 0) or (d == "b" and t == T - 1) \
                else None
            for m in range(8):
                o_m = bass.AP(tensor=sl.tensor,
                              offset=sl[:].offset + (m * SLOTW + toff) * BL,
                              ap=[sl[:].ap[0], [1, BL]])
                for k in range(2):
                    rhs = hin[:, k, :] if hin is not None else hist[d][:, k, tprev, :]
                    nc.tensor.matmul(o_m, lhsT=s_whh[d][:, k, m * 128:(m + 1) * 128],
                                     rhs=rhs, start=False, stop=False,
                                     skip_group_check=True)
            gin = bass.AP(tensor=sl.tensor,
                          offset=sl[:].offset + toff * BL,
                          ap=[sl[:].ap[0], [SLOTW * BL, 8], [1, BL]])
            s = tmp.tile([128, 8, BL], f32, tag=f"s{d}", name=f"s{d}")
            nc.scalar.activation(s[:], gin, AF.Sigmoid)
            for g in range(NG):
                cs = slice(g * GB, (g + 1) * GB)
                si, sf, sgt = s[:, 0:2, cs], s[:, 2:4, cs], s[:, 6:8, cs]
                ig = tmp.tile([128, 2, GB], f32, tag=f"ig{d}{g}", name=f"ig{d}{g}")
                nc.vector.scalar_tensor_tensor(ig[:], sgt, 0.5, si,
                                               op0=OP.subtract, op1=OP.mult)
                fc = tmp.tile([128, 2, GB], f32, tag=f"fc{d}{g}", name=f"fc{d}{g}")
                nc.gpsimd.tensor_tensor(fc[:], sf, c[:, :, cs], op=OP.mult)
                nc.vector.scalar_tensor_tensor(c[:, :, cs], ig[:], 2.0, fc[:],
                                               op0=OP.mult, op1=OP.add)
            th = tmp.tile([128, 2, BL], f32, tag=f"th{d}", name=f"th{d}")
            nc.scalar.activation(th[:], c[:], AF.Tanh)
            nc.vector.tensor_tensor(hist[d][:, :, t, :], s[:, 4:6, :], th[:],
                                    op=OP.mult)

        # ---------------- emission block assembly ----------------
        def emit_chunk(blk, q):
            """emit[:, blk*32+q*8 : .. +8, :] = woutF@hf + woutB@hb + bias."""
            t0 = blk * BLK + q * 8
            n = 8 * BL

            def work():
                pe = ps_s.tile([K, n], f32, tag="pssm", name="pe")
                for ci, d in ((0, "f"), (2, "b")):
                    for k in range(2):
                        nc.tensor.matmul(
                            pe[:], lhsT=s_wout[:, ci + k, :],
                            rhs=hist[d][:, k, t0:t0 + 8, :].rearrange("p t b -> p (t b)"),
                            start=(ci == 0 and k == 0), stop=(ci == 2 and k == 1))
                nc.scalar.activation(
                    emit[:, t0:t0 + 8, :].rearrange("k t b -> k (t b)"),
                    pe[:], AF.Identity, bias=s_bout[:, 0:1])
            return work

        def unary_piece(blk):
            c0 = blk * BLK * BL
            n = BLK * BL

            def work():
                src = emit[:, blk * BLK:(blk + 1) * BLK, :].rearrange("k t b -> k (t b)")
                t1 = tmp.tile([K, n], f32, tag="t1", name="t1", bufs=1)
                nc.scalar.activation(t1[:], s_t1f[:, c0:c0 + n], AF.Identity)
                um = tmp.tile([K, n], f32, tag="um", name="um", bufs=1)
                nc.gpsimd.tensor_tensor(um[:], t1[:], src, op=OP.mult)
                ur = tmp.tile([K, BL], f32, tag="ur", name="ur")
                umr = bass.AP(tensor=um.tensor, offset=um[:].offset,
                              ap=[um[:].ap[0], [1, BL], [BL, BLK]])
                nc.vector.tensor_reduce(ur[:], umr, axis=mybir.AxisListType.X, op=OP.add)
                nc.vector.tensor_tensor(Uacc[:], Uacc[:], ur[:], op=OP.add)
            return work

        # ---------------- CRF beta tail ----------------
        rescale_count = [0]

        def exp_block(blk):
            src = emit[:, blk * BLK:(blk + 1) * BLK, :].rearrange("k t b -> k (t b)")
            nc.scalar.activation(expE[:, blk % 2, :], src, AF.Exp)

        def _slot_ps(si, col0, parts, ncols):
            sl = slots[si]
            return bass.AP(tensor=sl.tensor, offset=sl[:].offset + col0,
                           ap=[[sl[:].ap[0][0], parts], [1, ncols]])

        def beta_step(s, sub):
            cs = slice(sub * SB, (sub + 1) * SB)
            blk = (s + 1) // BLK
            col = ((s + 1) % BLK) * BL + sub * SB
            bp = tmp.tile([K, SB], f32, tag=f"bp{sub}", name=f"bp{sub}")
            nc.gpsimd.tensor_tensor(bp[:], Bv[:, cs], expE[:, blk % 2, col:col + SB],
                                    op=OP.mult)
            # per-subgroup PSUM bank (window slots are dead in the tail)
            psb = _slot_ps(sub // 2, (sub % 2) * 512, K, SB)
            nc.tensor.matmul(psb, lhsT=s_expAT[:], rhs=bp[:], start=True, stop=True,
                             skip_group_check=True)
            nc.vector.copy_predicated(Bv[:, cs], maskrep[0:K, s + 1, cs], psb)

        def beta_rescale(sub):
            cs = slice(sub * SB, (sub + 1) * SB)
            pss = _slot_ps(2, (sub % 2) * 512 + (sub // 2) * 64, 1, SB)
            nc.tensor.matmul(pss, lhsT=onesf[0:K, 0:1], rhs=Bv[:, cs],
                             start=True, stop=True, skip_group_check=True)
            em = tmp.tile([1, SB], i32, tag=f"em{sub}", name=f"em{sub}")
            nc.vector.tensor_scalar(em[:], pss.bitcast(i32), c_mask[:, 0:1],
                                    None, op0=OP.bitwise_and)
            ef = tmp.tile([1, SB], f32, tag=f"ef{sub}", name=f"ef{sub}")
            nc.vector.tensor_copy(ef[:], em[:])
            nc.vector.scalar_tensor_tensor(Eacc[:, cs], ef[:], 1.0 / (1 << 23),
                                           Eacc[:, cs], op0=OP.mult, op1=OP.add)
            scf = tmp.tile([1, SB], f32, tag=f"scf{sub}", name=f"scf{sub}")
            nc.vector.tensor_scalar(scf[:], ef[:], -1.0, float(0x7F000000),
                                    op0=OP.mult, op1=OP.add)
            sci = tmp.tile([1, SB], i32, tag=f"sci{sub}", name=f"sci{sub}")
            nc.vector.tensor_copy(sci[:], scf[:])
            psr = _slot_ps(2, (sub % 2) * 512 + (sub // 2) * 64 + 16, K, SB)
            nc.tensor.matmul(psr, lhsT=onesf[0:1, 0:K], rhs=sci[:].bitcast(f32),
                             start=True, stop=True, skip_group_check=True)
            nc.vector.tensor_tensor(Bv[:, cs], Bv[:, cs], psr, op=OP.mult)
            rescale_count[0] += 1

        # ================ merged LSTM phase ================
        misc_q = []

        def drain_misc(n):
            while n > 0 and misc_q:
                misc_q.pop(0)()
                n -= 1

        # token gather cursors: fwd consumes chunks ascending, bwd descending
        lo, hi = [0], [NCH - 1]

        def need_lo(upto_col):
            while lo[0] * 128 < upto_col and lo[0] <= hi[0]:
                bg_q.append(gather_chunk(lo[0]))
                lo[0] += 1

        def need_hi(from_col):
            while (hi[0] + 1) * 128 > from_col and hi[0] >= lo[0]:
                bg_q.append(gather_chunk(hi[0]))
                hi[0] -= 1

        # block assembly readiness: blk fully covered at merged step
        # s >= max(blk*BLK+BLK-1, T-1-blk*BLK)
        ready_at = {}
        for blk in range(NBLK):
            ready_at.setdefault(max(blk * BLK + BLK - 1, T - 1 - blk * BLK),
                                []).append(blk)

        # prime: tokens + first windows (fw0, bw0, bw1)
        need_lo(fwin[0][1] * BL)
        need_hi(bwin[1][0] * BL)
        for p in build_pieces(0, "f"):
            bg_q.append(p)
        for p in build_pieces(0, "b"):
            bg_q.append(p)
        for p in build_pieces(1, "b"):
            bg_q.append(p)
        drain_bg(len(bg_q))

        tag_i = [0]
        for s in range(T):
            tf, tb = s, T - 1 - s
            if s >= 4 and s % 4 == 0 and s <= 504:
                k = (s - 4) // 4
                if k % 2 == 0:
                    j = k // 2 + 1
                    need_lo((fwin[j][0] + fwin[j][1]) * BL)
                    for p in build_pieces(j, "f"):
                        bg_q.append(p)
                else:
                    j = (k + 1) // 2 + 1
                    need_hi(bwin[j][0] * BL)
                    for p in build_pieces(j, "b"):
                        bg_q.append(p)
            if s % 8 == 0 and tag_i[0] < NCH:
                misc_q.append(tag_chunk(tag_i[0]))
                tag_i[0] += 1
            lstm_step("f", tf)
            lstm_step("b", tb)
            for blk in ready_at.get(s, []):
                for q in range(4):
                    misc_q.append(emit_chunk(blk, q))
                misc_q.append(unary_piece(blk))
            drain_bg(3)
            drain_misc(1)
        drain_bg(len(bg_q))
        drain_misc(len(misc_q))

        # ================ CRF beta tail ================
        exp_block(NBLK - 1)
        for s in range(T - 2, -1, -1):
            if (s + 1) % BLK == BLK - 1:
                exp_block((s + 1) // BLK)
            for sub in range(NSUB):
                beta_step(s, sub)
            if s % RESCALE == 0 and s > 0:
                for sub in range(NSUB):
                    beta_rescale(sub)

        # ================ finalize ================
        zt = fin.tile([K, BL], f32, tag="zt")
        nc.vector.tensor_tensor(zt[:], Bv[:], expE[:, 0, 0:BL], op=OP.mult)
        psz = ps_s.tile([1, BL], f32, tag="pssm", name="psz")
        nc.tensor.matmul(psz[:], lhsT=onesf[0:K, 0:1], rhs=zt[:], start=True, stop=True)
        logZ = fin.tile([1, BL], f32, tag="logZ")
        nc.scalar.activation(logZ[:], psz[:], AF.Ln)
        nc.vector.scalar_tensor_tensor(logZ[:], Eacc[:], float(np.log(2.0)), logZ[:],
                                       op0=OP.mult, op1=OP.add)
        nc.vector.tensor_scalar(
            logZ[:], logZ[:],
            float(-127.0 * (rescale_count[0] // NSUB) * np.log(2.0)), None,
            op0=OP.add)

        psu = ps_s.tile([1, BL], f32, tag="pssm", name="psu")
        nc.tensor.matmul(psu[:], lhsT=onesf[0:K, 0:1], rhs=Uacc[:], start=True, stop=True)
        score = fin.tile([1, BL], f32, tag="score")
        nc.vector.tensor_copy(score[:], psu[:])

        QT = T // 128
        pstr = ps_s.tile([1, NCH], f32, tag="pssm", name="pstr")
        nc.tensor.matmul(pstr[:], lhsT=onesf[:, 0:1], rhs=TRbuf[:], start=True, stop=True)
        trv = fin.tile([1, BL], f32, tag="trv")
        ptr_ap = bass.AP(tensor=pstr.tensor, offset=pstr[:].offset,
                         ap=[pstr[:].ap[0], [QT, BL], [1, QT]])
        nc.vector.tensor_reduce(trv[:], ptr_ap, axis=mybir.AxisListType.X, op=OP.add)

        dbg = fin.tile([1, 4 * BL], f32, tag="dbg")
        nc.vector.tensor_copy(dbg[:, 0 * BL:1 * BL], score[:])
        nc.vector.tensor_copy(dbg[:, 1 * BL:2 * BL], trv[:])
        nc.vector.tensor_copy(dbg[:, 2 * BL:3 * BL], logZ[:])
        nc.vector.tensor_copy(dbg[:, 3 * BL:4 * BL], Eacc[:])
        nc.sync.dma_start(out=out_dbg[:], in_=dbg[:])

        nc.vector.tensor_tensor(score[:], score[:], trv[:], op=OP.add)
        res = fin.tile([1, BL], f32, tag="res")
        nc.vector.tensor_tensor(res[:], logZ[:], score[:], op=OP.subtract)
        nc.sync.dma_start(out=out_loss[:], in_=res[:])

    nc.compile()
    return nc, names


def _prep_core(inputs, core, perm):
    import ml_dtypes
    bf = ml_dtypes.bfloat16
    s = slice(core * BL, (core + 1) * BL)
    sent = np.asarray(inputs["sentences"][s])
    tags = np.asarray(inputs["tags"][s])
    mask = (sent != PAD_IDX)
    maskT = mask.T
    toks = np.ascontiguousarray(sent.T).reshape(T * BL, 1)
    oh = (tags[:, :, None] == np.arange(K)[None, None, :])
    tags1h = (oh & mask[:, :, None]).transpose(2, 1, 0).reshape(K, T * BL)
    tnx = np.zeros((BL, T, K), np.float32)
    tnx[:, :-1, :] = (oh[:, 1:, :] & mask[:, 1:, None]).astype(np.float32)

    def wprep(wname):
        wt = np.asarray(inputs[wname], np.float32)[perm].copy()
        wt[6 * 128:, :] *= 2.0
        return np.ascontiguousarray(wt.T).astype(bf)

    bvec = {}
    for d, key in (("f", "b_f"), ("b", "b_b")):
        bb = np.asarray(inputs[key], np.float32)[perm].copy()
        bb[6 * 128:] *= 2.0
        bvec[d] = bb.reshape(1, 4 * H).astype(bf)

    return {
        "toks": toks.astype(np.int32),
        "masku": maskT.astype(np.uint8).reshape(1, T * BL),
        "negm": ((~maskT).astype(np.float32) * -1e5).reshape(1, T * BL).astype(bf),
        "tags1f": tags1h.astype(np.uint8),
        "tagsnx": tnx.reshape(T * BL, K).astype(np.uint8),
        "tagsfl": tags.reshape(T * BL, 1).astype(np.int32),
        "emb": np.asarray(inputs["embedding"], np.float32).astype(bf),
        "wih_f": wprep("w_ih_f"), "wih_b": wprep("w_ih_b"),
        "whh_f": wprep("w_hh_f"), "whh_b": wprep("w_hh_b"),
        "brow_f": bvec["f"], "brow_b": bvec["b"],
        "woutT": np.ascontiguousarray(
            np.asarray(inputs["w_out"], np.float32).T.reshape(4, 128, K)).astype(bf),
        "bout": np.asarray(inputs["b_out"]).reshape(K, 1).astype(np.float32),
        "transT": np.ascontiguousarray(np.asarray(inputs["transition"]).T).astype(np.float32),
        "trans": np.asarray(inputs["transition"], np.float32),
    }


def kernel(**inputs):
    from concourse.bass_utils import run_bass_kernel_spmd

    if "prog" not in _cache:
        _cache["prog"] = _build_program()
    nc, names = _cache["prog"]

    blocks = np.arange(4 * H).reshape(4, H)
    perm = np.concatenate([blocks[0], blocks[1], blocks[3], blocks[2]])

    in_maps = []
    for core in range(NCORES):
        m = _prep_core(inputs, core, perm)
        in_maps.append({names[kk]: vv for kk, vv in m.items()})

    res = run_bass_kernel_spmd(nc, in_maps, core_ids=list(range(NCORES)),
                               **_cache.get("run_kwargs", {}))
    out = np.concatenate([r[names["out"]].reshape(BL) for r in res.results])
    _cache["last_results"] = res
    if "dbg" in names:
        _cache["dbg"] = np.concatenate(
            [r[names["dbg"]].reshape(4, BL) for r in res.results], axis=1)
    return out.astype(np.float32)


# revision 27
# speedup vs baseline: 1.2014x; 1.0442x over previous
"""BiLSTM-CRF loss kernel for Trainium2 (8 NeuronCores, data-parallel over batch).

v3: merged-direction LSTM phase + separate pipelined CRF tail.
  - Both LSTM directions run interleaved in ONE loop: two independent
    dependency chains pipeline across PE/Act/DVE/Pool, hiding the per-step
    serial latency that bounded v2.
  - Hidden state written straight into per-direction h histories (bf16);
    emissions are assembled in batched 32-step blocks (4 matmuls + one
    scalar-engine bias pass per 128-col chunk) once both directions have
    covered the block - no per-step emission work at all.
  - Input projection per 4-step window into 1-bank PSUM slots (2 per
    direction, ping-pong); gate bias and the bwd -1e5 pad-kill folded in as
    rank-1 matmuls; recurrence matmuls accumulate in place (start=False).
  - Activations: one sigmoid over all 8 gate chunks per group (g-rows
    pre-doubled; tanh(x)=2*sigmoid(2x)-1 fixed in cell math) + tanh(c) per
    group; sigmoid+tanh share one act table -> no table loads in the loop.
  - Forward direction unmasked (pad suffix garbage is bounded and never
    read); backward masked via the -1e5 gate injection (h=c=0 exactly).
  - CRF beta recursion in exp space as a tail, 2 column-subgroups
    pipelined; exp(emit) batched per block (sigmoid never used in the tail
    -> one act-table load total); rescale every 8 steps via fp32 exponent
    bit extraction (no Ln / reciprocal).
"""

import numpy as np

PAD_IDX = 0
VOCAB, K, E, H = 30000, 20, 256, 256
B, T = 128, 512
NCORES = 8
BL = B // NCORES          # 16 sequences per core
NG = 2                    # batch groups per direction
GB = BL // NG
SLOTW = 8                 # slot capacity (steps); slot = 2 psum banks
BLK = 32                  # emit/exp/unary block size
NBLK = T // BLK
RESCALE = 8               # CRF rescale interval (beta steps, per subgroup)
NSUB = 2                  # CRF column subgroups
SB = BL // NSUB
NCH = 64                  # 128-token gather chunks

_cache = {}


def _build_program():
    from contextlib import ExitStack
    import concourse.bass as bass
    import concourse.bacc as bacc
    import concourse.tile as tile
    from concourse import mybir
    from concourse.masks import make_identity

    f32 = mybir.dt.float32
    i32 = mybir.dt.int32
    bf16 = mybir.dt.bfloat16
    u8 = mybir.dt.uint8
    AF = mybir.ActivationFunctionType
    OP = mybir.AluOpType

    nc = bacc.Bacc(None, target_bir_lowering=False, debug=False)
    names = {}

    with ExitStack() as ctx:
        tc = ctx.enter_context(tile.TileContext(nc))
        dram = ctx.enter_context(tc.tile_pool(name="dram", bufs=1, space="DRAM"))

        def din(key, shape, dt=f32):
            t = dram.tile(shape, dt, kind="ExternalInput", name=key)
            names[key] = t.tensor.name
            return t

        emb = din("emb", [VOCAB, E], bf16)
        toks = din("toks", [T * BL, 1], i32)
        masku = din("masku", [1, T * BL], u8)
        negm = din("negm", [1, T * BL], bf16)
        tags1f = din("tags1f", [K, T * BL], u8)
        tagsnx = din("tagsnx", [T * BL, K], u8)
        tagsfl = din("tagsfl", [T * BL, 1], i32)
        wih = {d: din(f"wih_{d}", [E, 4 * H], bf16) for d in "fb"}
        whh = {d: din(f"whh_{d}", [E, 4 * H], bf16) for d in "fb"}
        brow = {d: din(f"brow_{d}", [1, 4 * H], bf16) for d in "fb"}
        woutT = din("woutT", [4, 128, K], bf16)
        bout = din("bout", [K, 1])
        transT = din("transT", [K, K])
        trans = din("trans", [K, K])
        out_loss = dram.tile([1, BL], f32, kind="ExternalOutput")
        names["out"] = out_loss.tensor.name
        out_dbg = dram.tile([1, 4 * BL], f32, kind="ExternalOutput", name="out_dbg")
        names["dbg"] = out_dbg.tensor.name

        # PSUM: 4 window slots (1 bank each) + transpose + small tiles
        ps_slot = ctx.enter_context(tc.tile_pool(name="ps_slot", bufs=1, space="PSUM"))
        ps_t = ctx.enter_context(tc.tile_pool(name="ps_t", bufs=1, space="PSUM"))
        ps_s = ctx.enter_context(tc.tile_pool(name="ps_s", bufs=1, space="PSUM"))

        sg = ctx.enter_context(tc.tile_pool(name="sg", bufs=1))
        tmp = ctx.enter_context(tc.tile_pool(name="tmp", bufs=4))
        gat = ctx.enter_context(tc.tile_pool(name="gat", bufs=4))
        neg = ctx.enter_context(tc.tile_pool(name="neg", bufs=2))
        fin = ctx.enter_context(tc.tile_pool(name="fin", bufs=3))

        slots = [ps_slot.tile([128, 8, SLOTW, BL], f32, tag=f"slot{i}",
                              name=f"slot{i}") for i in range(3)]

        # window schedule: fwd = 64 x 8-step windows; bwd = 4-step head,
        # 63 x 8-step, 4-step tail -> boundaries stagger every 4 merged steps
        fwin = [(8 * i, 8) for i in range(64)]
        bwin = [(508, 4)] + [(500 - 8 * i, 8) for i in range(63)] + [(0, 4)]
        fslot = [0] * len(fwin)
        bslot = [0] * len(bwin)
        bslot[0], bslot[1] = 1, 2
        rot = [1, 0, 2]
        for k in range(126):
            sl = rot[k % 3]
            if k % 2 == 0:
                fslot[k // 2 + 1] = sl
            else:
                bslot[(k + 1) // 2 + 1] = sl
        # per-t lookup: (window index, toff)
        fmap = [None] * T
        for j, (t0, ln) in enumerate(fwin):
            for o in range(ln):
                fmap[t0 + o] = (j, o)
        bmap = [None] * T
        for j, (t0, ln) in enumerate(bwin):
            for o in range(ln):
                bmap[t0 + o] = (j, o)

        # ---- resident SBUF tensors ----
        s_wih = {d: sg.tile([128, 2, 4 * H], bf16, tag=f"wih{d}", name=f"wih{d}")
                 for d in "fb"}
        s_whh = {d: sg.tile([128, 2, 4 * H], bf16, tag=f"whh{d}", name=f"whh{d}")
                 for d in "fb"}
        s_brow = {d: sg.tile([1, 4 * H], bf16, tag=f"brow{d}", name=f"brow{d}")
                  for d in "fb"}
        for d in "fb":
            nc.sync.dma_start(out=s_wih[d][:], in_=wih[d][:].rearrange("(k p) m -> p k m", p=128))
            nc.sync.dma_start(out=s_whh[d][:], in_=whh[d][:].rearrange("(k p) m -> p k m", p=128))
            nc.sync.dma_start(out=s_brow[d][:], in_=brow[d][:])
        s_wout = sg.tile([128, 4, K], bf16, tag="wout")
        nc.sync.dma_start(out=s_wout[:], in_=woutT[:].rearrange("c p k -> p c k"))
        s_bout = sg.tile([K, 1], f32, tag="bout")
        nc.sync.dma_start(out=s_bout[:], in_=bout[:])
        s_transT = sg.tile([K, K], f32, tag="transT")
        nc.sync.dma_start(out=s_transT[:], in_=transT[:])
        s_expAT = sg.tile([K, K], f32, tag="expAT")
        nc.scalar.activation(s_expAT[:], s_transT[:], AF.Exp)
        s_expATb = sg.tile([K, K], bf16, tag="expATb")
        nc.vector.tensor_copy(s_expATb[:], s_expAT[:])

        onesb = sg.tile([1, 256], bf16, tag="onesb")
        nc.vector.memset(onesb[:], 1.0)
        onesrow = sg.tile([1, 128], bf16, tag="onesrow")
        nc.vector.memset(onesrow[:], 1.0)
        onesf = sg.tile([128, K], f32, tag="onesf")
        nc.vector.memset(onesf[:], 1.0)
        ident = sg.tile([128, 128], bf16, tag="ident")
        make_identity(nc, ident[:])

        c_mask = sg.tile([1, 1], i32, tag="c_mask")
        nc.vector.memset(c_mask[:], 0x7F800000)

        maskrep = sg.tile([128, T, BL], u8, tag="maskrep")
        nc.sync.dma_start(
            out=maskrep[:],
            in_=bass.AP(tensor=masku.tensor, offset=masku[:].offset,
                        ap=[[0, 128], [BL, T], [1, BL]]))

        idxall = sg.tile([128, NCH], i32, tag="idxall")
        nc.sync.dma_start(out=idxall[:],
                          in_=bass.AP(tensor=toks.tensor, offset=toks[:].offset,
                                      ap=[[1, 128], [128, NCH]]))
        idxtag = sg.tile([128, NCH], i32, tag="idxtag")
        nc.sync.dma_start(out=idxtag[:],
                          in_=bass.AP(tensor=tagsfl.tensor, offset=tagsfl[:].offset,
                                      ap=[[1, 128], [128, NCH]]))
        s_t1f = sg.tile([K, T * BL], u8, tag="s_t1f")
        nc.sync.dma_start(out=s_t1f[:], in_=tags1f[:])
        s_tnx = sg.tile([128, NCH, K], u8, tag="s_tnx")
        nc.sync.dma_start(out=s_tnx[:],
                          in_=tagsnx[:].rearrange("(n p) k -> p n k", p=128))

        xT = sg.tile([128, 2, T * BL], bf16, tag="xT")
        emit = sg.tile([K, T, BL], f32, tag="emit")
        expE = sg.tile([K, 2, BLK * BL], f32, tag="expE")
        hist = {d: sg.tile([128, 2, T, BL], bf16, tag=f"hist{d}", name=f"hist{d}")
                for d in "fb"}
        hzero = sg.tile([128, 2, BL], bf16, tag="hzero")
        nc.vector.memset(hzero[:], 0.0)

        st_c = {d: sg.tile([128, 2, BL], f32, tag=f"c{d}", name=f"c{d}") for d in "fb"}
        for d in "fb":
            nc.vector.memset(st_c[d][:], 0.0)

        Bv = sg.tile([K, BL], f32, tag="Bv")
        nc.vector.memset(Bv[:], 1.0)
        Eacc = sg.tile([1, BL], f32, tag="Eacc")
        nc.vector.memset(Eacc[:], 0.0)
        Uacc = sg.tile([K, BL], f32, tag="Uacc")
        nc.vector.memset(Uacc[:], 0.0)
        TRbuf = sg.tile([128, NCH], f32, tag="TRbuf")

        # ---- warm-up matmuls ----
        for wt in [s_wih["f"][:, 0, 0:1], s_wih["b"][:, 0, 0:1],
                   s_whh["f"][:, 0, 0:1], s_whh["b"][:, 0, 0:1],
                   s_wout[:, 0, 0:1], ident[:, 0:1]]:
            psd = ps_s.tile([1, 1], f32, tag="pssm", name="psd")
            nc.tensor.matmul(psd[:], lhsT=wt, rhs=wt, start=True, stop=True)
        psd = ps_s.tile([1, 1], f32, tag="pssm", name="psd")
        nc.tensor.matmul(psd[:], lhsT=s_expAT[0:K, 0:1], rhs=s_expAT[0:K, 0:1],
                         start=True, stop=True)

        # ---------------- background work queue ----------------
        bg_q = []

        def drain_bg(n):
            while n > 0 and bg_q:
                bg_q.pop(0)()
                n -= 1

        gathered = [0]

        def gather_chunk(c):
            def work():
                xg = gat.tile([128, E], bf16, tag="xg", name="xg")
                nc.gpsimd.indirect_dma_start(
                    out=xg[:], out_offset=None, in_=emb[:],
                    in_offset=bass.IndirectOffsetOnAxis(ap=idxall[:, c:c + 1], axis=0))
                for k in range(2):
                    pst = ps_t.tile([128, 128], bf16, tag="pst", name="pst")
                    nc.tensor.transpose(out=pst[:], in_=xg[:, k * 128:(k + 1) * 128],
                                        identity=ident[:])
                    nc.vector.tensor_copy(xT[:, k, c * 128:(c + 1) * 128], pst[:])
            return work

        def tag_chunk(i):
            def work():
                tr = gat.tile([128, K], f32, tag="tr", name="tr")
                nc.gpsimd.indirect_dma_start(
                    out=tr[:], out_offset=None, in_=trans[:],
                    in_offset=bass.IndirectOffsetOnAxis(ap=idxtag[:, i:i + 1], axis=0))
                sel = gat.tile([128, K], f32, tag="sel", name="sel")
                nc.vector.tensor_copy(sel[:], s_tnx[:, i, :])
                nc.vector.tensor_tensor(tr[:], tr[:], sel[:], op=OP.mult)
                nc.vector.tensor_reduce(TRbuf[:, i:i + 1], tr[:],
                                        axis=mybir.AxisListType.X, op=OP.add)
            return work

        # ---------------- window build pieces ----------------
        def negdma_piece(t0, ln, nm):
            c0, ncol = t0 * BL, ln * BL

            def work():
                nc.sync.dma_start(out=nm[:, 0:ncol], in_=negm[:, c0:c0 + ncol])
            return work

        def proj_piece(t0, ln, si, d, m, nm):
            c0, ncol = t0 * BL, ln * BL
            sl = slots[si]

            def work():
                o_m = bass.AP(tensor=sl.tensor,
                              offset=sl[:].offset + m * SLOTW * BL,
                              ap=[sl[:].ap[0], [1, ncol]])
                for k in range(2):
                    nc.tensor.matmul(o_m, lhsT=s_wih[d][:, k, m * 128:(m + 1) * 128],
                                     rhs=xT[:, k, c0:c0 + ncol],
                                     start=(m in (0, 4) and k == 0), stop=False,
                                     skip_group_check=True)
                nc.tensor.matmul(o_m, lhsT=s_brow[d][:, m * 128:(m + 1) * 128],
                                 rhs=onesb[:, 0:ncol], start=False, stop=False,
                                 skip_group_check=True)
                if d == "b" and m < 6:
                    nc.tensor.matmul(o_m, lhsT=onesrow[:, 0:128],
                                     rhs=nm[:, 0:ncol], start=False, stop=False,
                                     skip_group_check=True)
            return work

        def build_pieces(j, d):
            t0, ln = (fwin if d == "f" else bwin)[j]
            si = (fslot if d == "f" else bslot)[j]
            pieces = []
            nm = None
            if d == "b":
                nm = neg.tile([1, 256], bf16, tag="nm", name="nm")
                pieces.append(negdma_piece(t0, ln, nm))
            for m in range(8):
                pieces.append(proj_piece(t0, ln, si, d, m, nm))
            return pieces

        # ---------------- LSTM step ----------------
        def lstm_step(d, t):
            j, toff = (fmap if d == "f" else bmap)[t]
            sl = slots[(fslot if d == "f" else bslot)[j]]
            c = st_c[d]
            tprev = t - 1 if d == "f" else t + 1
            hin = hzero if (d == "f" and t == 0) or (d == "b" and t == T - 1) \
                else None
            for m in range(8):
                o_m = bass.AP(tensor=sl.tensor,
                              offset=sl[:].offset + (m * SLOTW + toff) * BL,
                              ap=[sl[:].ap[0], [1, BL]])
                for k in range(2):
                    rhs = hin[:, k, :] if hin is not None else hist[d][:, k, tprev, :]
                    nc.tensor.matmul(o_m, lhsT=s_whh[d][:, k, m * 128:(m + 1) * 128],
                                     rhs=rhs, start=False, stop=False,
                                     skip_group_check=True)
            gin = bass.AP(tensor=sl.tensor,
                          offset=sl[:].offset + toff * BL,
                          ap=[sl[:].ap[0], [SLOTW * BL, 8], [1, BL]])
            s = tmp.tile([128, 8, BL], f32, tag=f"s{d}", name=f"s{d}")
            nc.scalar.activation(s[:], gin, AF.Sigmoid)
            for g in range(NG):
                cs = slice(g * GB, (g + 1) * GB)
                si, sf, sgt = s[:, 0:2, cs], s[:, 2:4, cs], s[:, 6:8, cs]
                ig = tmp.tile([128, 2, GB], f32, tag=f"ig{d}{g}", name=f"ig{d}{g}")
                nc.vector.scalar_tensor_tensor(ig[:], sgt, 0.5, si,
                                               op0=OP.subtract, op1=OP.mult)
                fc = tmp.tile([128, 2, GB], f32, tag=f"fc{d}{g}", name=f"fc{d}{g}")
                nc.gpsimd.tensor_tensor(fc[:], sf, c[:, :, cs], op=OP.mult)
                nc.vector.scalar_tensor_tensor(c[:, :, cs], ig[:], 2.0, fc[:],
                                               op0=OP.mult, op1=OP.add)
            th = tmp.tile([128, 2, BL], f32, tag=f"th{d}", name=f"th{d}")
            nc.scalar.activation(th[:], c[:], AF.Tanh)
            nc.vector.tensor_tensor(hist[d][:, :, t, :], s[:, 4:6, :], th[:],
                                    op=OP.mult)

        # ---------------- emission block assembly ----------------
        def emit_chunk(blk, q):
            """emit[:, blk*32+q*8 : .. +8, :] = woutF@hf + woutB@hb + bias."""
            t0 = blk * BLK + q * 8
            n = 8 * BL

            def work():
                pe = ps_s.tile([K, n], f32, tag="pssm", name="pe")
                for ci, d in ((0, "f"), (2, "b")):
                    for k in range(2):
                        nc.tensor.matmul(
                            pe[:], lhsT=s_wout[:, ci + k, :],
                            rhs=hist[d][:, k, t0:t0 + 8, :].rearrange("p t b -> p (t b)"),
                            start=(ci == 0 and k == 0), stop=(ci == 2 and k == 1))
                nc.scalar.activation(
                    emit[:, t0:t0 + 8, :].rearrange("k t b -> k (t b)"),
                    pe[:], AF.Identity, bias=s_bout[:, 0:1])
            return work

        def unary_piece(blk):
            c0 = blk * BLK * BL
            n = BLK * BL

            def work():
                src = emit[:, blk * BLK:(blk + 1) * BLK, :].rearrange("k t b -> k (t b)")
                t1 = tmp.tile([K, n], f32, tag="t1", name="t1", bufs=1)
                nc.scalar.activation(t1[:], s_t1f[:, c0:c0 + n], AF.Identity)
                um = tmp.tile([K, n], f32, tag="um", name="um", bufs=1)
                nc.gpsimd.tensor_tensor(um[:], t1[:], src, op=OP.mult)
                ur = tmp.tile([K, BL], f32, tag="ur", name="ur")
                umr = bass.AP(tensor=um.tensor, offset=um[:].offset,
                              ap=[um[:].ap[0], [1, BL], [BL, BLK]])
                nc.vector.tensor_reduce(ur[:], umr, axis=mybir.AxisListType.X, op=OP.add)
                nc.vector.tensor_tensor(Uacc[:], Uacc[:], ur[:], op=OP.add)
            return work

        # ---------------- CRF beta tail ----------------
        rescale_count = [0]

        def exp_block(blk):
            src = emit[:, blk * BLK:(blk + 1) * BLK, :].rearrange("k t b -> k (t b)")
            nc.scalar.activation(expE[:, blk % 2, :], src, AF.Exp)

        def _slot_ps(si, col0, parts, ncols):
            sl = slots[si]
            return bass.AP(tensor=sl.tensor, offset=sl[:].offset + col0,
                           ap=[[sl[:].ap[0][0], parts], [1, ncols]])

        def beta_step(s, sub):
            cs = slice(sub * SB, (sub + 1) * SB)
            blk = (s + 1) // BLK
            col = ((s + 1) % BLK) * BL + sub * SB
            bp = tmp.tile([K, SB], bf16, tag=f"bp{sub}", name=f"bp{sub}")
            nc.gpsimd.tensor_tensor(bp[:], Bv[:, cs], expE[:, blk % 2, col:col + SB],
                                    op=OP.mult)
            # per-subgroup PSUM bank (window slots are dead in the tail)
            psb = _slot_ps(sub // 2, (sub % 2) * 512, K, SB)
            nc.tensor.matmul(psb, lhsT=s_expATb[:], rhs=bp[:], start=True, stop=True,
                             skip_group_check=True)
            nc.vector.copy_predicated(Bv[:, cs], maskrep[0:K, s + 1, cs], psb)

        def beta_rescale(sub):
            cs = slice(sub * SB, (sub + 1) * SB)
            pss = _slot_ps(2, (sub % 2) * 512 + (sub // 2) * 64, 1, SB)
            nc.tensor.matmul(pss, lhsT=onesf[0:K, 0:1], rhs=Bv[:, cs],
                             start=True, stop=True, skip_group_check=True)
            em = tmp.tile([1, SB], i32, tag=f"em{sub}", name=f"em{sub}")
            nc.vector.tensor_scalar(em[:], pss.bitcast(i32), c_mask[:, 0:1],
                                    None, op0=OP.bitwise_and)
            ef = tmp.tile([1, SB], f32, tag=f"ef{sub}", name=f"ef{sub}")
            nc.vector.tensor_copy(ef[:], em[:])
            nc.vector.scalar_tensor_tensor(Eacc[:, cs], ef[:], 1.0 / (1 << 23),
                                           Eacc[:, cs], op0=OP.mult, op1=OP.add)
            scf = tmp.tile([1, SB], f32, tag=f"scf{sub}", name=f"scf{sub}")
            nc.vector.tensor_scalar(scf[:], ef[:], -1.0, float(0x7F000000),
                                    op0=OP.mult, op1=OP.add)
            sci = tmp.tile([1, SB], i32, tag=f"sci{sub}", name=f"sci{sub}")
            nc.vector.tensor_copy(sci[:], scf[:])
            psr = _slot_ps(2, (sub % 2) * 512 + (sub // 2) * 64 + 16, K, SB)
            nc.tensor.matmul(psr, lhsT=onesf[0:1, 0:K], rhs=sci[:].bitcast(f32),
                             start=True, stop=True, skip_group_check=True)
            nc.vector.tensor_tensor(Bv[:, cs], Bv[:, cs], psr, op=OP.mult)
            rescale_count[0] += 1

        # ================ merged LSTM phase ================
        misc_q = []

        def drain_misc(n):
            while n > 0 and misc_q:
                misc_q.pop(0)()
                n -= 1

        # token gather cursors: fwd consumes chunks ascending, bwd descending
        lo, hi = [0], [NCH - 1]

        def need_lo(upto_col):
            while lo[0] * 128 < upto_col and lo[0] <= hi[0]:
                bg_q.append(gather_chunk(lo[0]))
                lo[0] += 1

        def need_hi(from_col):
            while (hi[0] + 1) * 128 > from_col and hi[0] >= lo[0]:
                bg_q.append(gather_chunk(hi[0]))
                hi[0] -= 1

        # block assembly readiness: blk fully covered at merged step
        # s >= max(blk*BLK+BLK-1, T-1-blk*BLK)
        ready_at = {}
        for blk in range(NBLK):
            ready_at.setdefault(max(blk * BLK + BLK - 1, T - 1 - blk * BLK),
                                []).append(blk)

        # prime: tokens + first windows (fw0, bw0, bw1)
        need_lo(fwin[0][1] * BL)
        need_hi(bwin[1][0] * BL)
        for p in build_pieces(0, "f"):
            bg_q.append(p)
        for p in build_pieces(0, "b"):
            bg_q.append(p)
        for p in build_pieces(1, "b"):
            bg_q.append(p)
        drain_bg(len(bg_q))

        tag_i = [0]
        for s in range(T):
            tf, tb = s, T - 1 - s
            if s >= 4 and s % 4 == 0 and s <= 504:
                k = (s - 4) // 4
                if k % 2 == 0:
                    j = k // 2 + 1
                    need_lo((fwin[j][0] + fwin[j][1]) * BL)
                    for p in build_pieces(j, "f"):
                        bg_q.append(p)
                else:
                    j = (k + 1) // 2 + 1
                    need_hi(bwin[j][0] * BL)
                    for p in build_pieces(j, "b"):
                        bg_q.append(p)
            if s % 8 == 0 and tag_i[0] < NCH:
                misc_q.append(tag_chunk(tag_i[0]))
                tag_i[0] += 1
            lstm_step("f", tf)
            lstm_step("b", tb)
            for blk in ready_at.get(s, []):
                for q in range(4):
                    misc_q.append(emit_chunk(blk, q))
                misc_q.append(unary_piece(blk))
            drain_bg(3)
            drain_misc(1)
        drain_bg(len(bg_q))
        drain_misc(len(misc_q))

        # ================ CRF beta tail ================
        exp_block(NBLK - 1)
        for s in range(T - 2, -1, -1):
            if (s + 1) % BLK == BLK - 1:
                exp_block((s + 1) // BLK)
            for sub in range(NSUB):
                beta_step(s, sub)
            if s % RESCALE == 0 and s > 0:
                for sub in range(NSUB):
                    beta_rescale(sub)

        # ================ finalize ================
        zt = fin.tile([K, BL], f32, tag="zt")
        nc.vector.tensor_tensor(zt[:], Bv[:], expE[:, 0, 0:BL], op=OP.mult)
        psz = ps_s.tile([1, BL], f32, tag="pssm", name="psz")
        nc.tensor.matmul(psz[:], lhsT=onesf[0:K, 0:1], rhs=zt[:], start=True, stop=True)
        logZ = fin.tile([1, BL], f32, tag="logZ")
        nc.scalar.activation(logZ[:], psz[:], AF.Ln)
        nc.vector.scalar_tensor_tensor(logZ[:], Eacc[:], float(np.log(2.0)), logZ[:],
                                       op0=OP.mult, op1=OP.add)
        nc.vector.tensor_scalar(
            logZ[:], logZ[:],
            float(-127.0 * (rescale_count[0] // NSUB) * np.log(2.0)), None,
            op0=OP.add)

        psu = ps_s.tile([1, BL], f32, tag="pssm", name="psu")
        nc.tensor.matmul(psu[:], lhsT=onesf[0:K, 0:1], rhs=Uacc[:], start=True, stop=True)
        score = fin.tile([1, BL], f32, tag="score")
        nc.vector.tensor_copy(score[:], psu[:])

        QT = T // 128
        pstr = ps_s.tile([1, NCH], f32, tag="pssm", name="pstr")
        nc.tensor.matmul(pstr[:], lhsT=onesf[:, 0:1], rhs=TRbuf[:], start=True, stop=True)
        trv = fin.tile([1, BL], f32, tag="trv")
        ptr_ap = bass.AP(tensor=pstr.tensor, offset=pstr[:].offset,
                         ap=[pstr[:].ap[0], [QT, BL], [1, QT]])
        nc.vector.tensor_reduce(trv[:], ptr_ap, axis=mybir.AxisListType.X, op=OP.add)

        dbg = fin.tile([1, 4 * BL], f32, tag="dbg")
        nc.vector.tensor_copy(dbg[:, 0 * BL:1 * BL], score[:])
        nc.vector.tensor_copy(dbg[:, 1 * BL:2 * BL], trv[:])
        nc.vector.tensor_copy(dbg[:, 2 * BL:3 * BL], logZ[:])
        nc.vector.tensor_copy(dbg[:, 3 * BL:4 * BL], Eacc[:])
        nc.sync.dma_start(out=out_dbg[:], in_=dbg[:])

        nc.vector.tensor_tensor(score[:], score[:], trv[:], op=OP.add)
        res = fin.tile([1, BL], f32, tag="res")
        nc.vector.tensor_tensor(res[:], logZ[:], score[:], op=OP.subtract)
        nc.sync.dma_start(out=out_loss[:], in_=res[:])

    nc.compile()
    return nc, names


def _prep_core(inputs, core, perm):
    import ml_dtypes
    bf = ml_dtypes.bfloat16
    s = slice(core * BL, (core + 1) * BL)
    sent = np.asarray(inputs["sentences"][s])
    tags = np.asarray(inputs["tags"][s])
    mask = (sent != PAD_IDX)
    maskT = mask.T
    toks = np.ascontiguousarray(sent.T).reshape(T * BL, 1)
    oh = (tags[:, :, None] == np.arange(K)[None, None, :])
    tags1h = (oh & mask[:, :, None]).transpose(2, 1, 0).reshape(K, T * BL)
    tnx = np.zeros((BL, T, K), np.float32)
    tnx[:, :-1, :] = (oh[:, 1:, :] & mask[:, 1:, None]).astype(np.float32)

    def wprep(wname):
        wt = np.asarray(inputs[wname], np.float32)[perm].copy()
        wt[6 * 128:, :] *= 2.0
        return np.ascontiguousarray(wt.T).astype(bf)

    bvec = {}
    for d, key in (("f", "b_f"), ("b", "b_b")):
        bb = np.asarray(inputs[key], np.float32)[perm].copy()
        bb[6 * 128:] *= 2.0
        bvec[d] = bb.reshape(1, 4 * H).astype(bf)

    return {
        "toks": toks.astype(np.int32),
        "masku": maskT.astype(np.uint8).reshape(1, T * BL),
        "negm": ((~maskT).astype(np.float32) * -1e5).reshape(1, T * BL).astype(bf),
        "tags1f": tags1h.astype(np.uint8),
        "tagsnx": tnx.reshape(T * BL, K).astype(np.uint8),
        "tagsfl": tags.reshape(T * BL, 1).astype(np.int32),
        "emb": np.asarray(inputs["embedding"], np.float32).astype(bf),
        "wih_f": wprep("w_ih_f"), "wih_b": wprep("w_ih_b"),
        "whh_f": wprep("w_hh_f"), "whh_b": wprep("w_hh_b"),
        "brow_f": bvec["f"], "brow_b": bvec["b"],
        "woutT": np.ascontiguousarray(
            np.asarray(inputs["w_out"], np.float32).T.reshape(4, 128, K)).astype(bf),
        "bout": np.asarray(inputs["b_out"]).reshape(K, 1).astype(np.float32),
        "transT": np.ascontiguousarray(np.asarray(inputs["transition"]).T).astype(np.float32),
        "trans": np.asarray(inputs["transition"], np.float32),
    }


def kernel(**inputs):
    from concourse.bass_utils import run_bass_kernel_spmd

    if "prog" not in _cache:
        _cache["prog"] = _build_program()
    nc, names = _cache["prog"]

    blocks = np.arange(4 * H).reshape(4, H)
    perm = np.concatenate([blocks[0], blocks[1], blocks[3], blocks[2]])

    in_maps = []
    for core in range(NCORES):
        m = _prep_core(inputs, core, perm)
        in_maps.append({names[kk]: vv for kk, vv in m.items()})

    res = run_bass_kernel_spmd(nc, in_maps, core_ids=list(range(NCORES)),
                               **_cache.get("run_kwargs", {}))
    out = np.concatenate([r[names["out"]].reshape(BL) for r in res.results])
    _cache["last_results"] = res
    if "dbg" in names:
        _cache["dbg"] = np.concatenate(
            [r[names["dbg"]].reshape(4, BL) for r in res.results], axis=1)
    return out.astype(np.float32)
